# revision 11
# baseline (speedup 1.0000x reference)
"""GPTBigCode MQA causal attention block on 8 TRN2 NeuronCores — v4.

Tensor-parallel over heads (4 of 32 query heads per core, single KV head
replicated), row-parallel c_proj, bf16 partial outputs summed on host.

v4 = v3 (fp8 DoubleRow GEMMs + fp8 off-diag attention) plus:
- Scores in fp8 DoubleRow at 0.5x: K^T is plain fp8 (x16) duplicated
  across both stationary slots via a stride-0 AP; Q is split hi+lo fp8
  (x16) in the two moving slots, so one DoubleRow computes
  k8^T(q_hi+q_lo) — full Q precision, only K carries plain-fp8 error.
- Engine rebalance (GPSIMD cannot touch PSUM; DVE's 2x path needs all
  operands 2-byte): y evictions alternate DVE/ACT; SBUF-only fp8 ops
  (prob pair combine, q/v/at lo-splits) go to the idle GPSIMD; the
  softmax-denominator accumulation chain is fp16-only on DVE (2x mode)
  fed by per-pair GPSIMD combines that run in parallel.
"""

import numpy as np
from contextlib import ExitStack

import ml_dtypes
import concourse.bass as bass
import concourse.tile as tile
from concourse import bass_isa, mybir
from concourse.bass_utils import run_bass_kernel_spmd
from concourse.masks import make_identity

B, S, D = 2, 2048, 4096
H, DH = 32, 128
NCORES = 8
HC = H // NCORES          # 4 heads per core
DQC = HC * DH             # 512 q-dims per core
T = B * S                 # 4096 tokens
P = 128
NKD = D // P              # 32 contraction tiles in model dim
E1 = DQC + 2 * DH         # 768 per-core QKV output dims
NEB = E1 // P             # 6 e-blocks: 4 Q heads, K, V
QT = 512                  # tokens per (b,j) group
NJ = T // QT              # 8 groups
NJB = S // QT             # 4 groups per batch
SCALE = DH ** -0.5
NKC = NKD // 2            # kd tiles per xq chunk

SX = 32.0                 # x fp8 scale
SW = 2048.0               # weight fp8 scale
SV = 16.0                 # q/k/v scale (fp16 master + fp8)
DEQ = 1.0 / (SX * SW)     # 2^-16
EXPB = -2.0               # exp bias: p~ = e^(s*SCALE - 2)
NQKV = 48                 # DoubleRow instrs per QKV e-block

F32 = mybir.dt.float32
R32 = mybir.dt.float32r
BF16 = mybir.dt.bfloat16
F16 = mybir.dt.float16
FP8 = mybir.dt.float8e4
ACTF = mybir.ActivationFunctionType
DR = mybir.MatmulPerfMode.DoubleRow
ALU = mybir.AluOpType
NEG = -1.0e30
BF = ml_dtypes.bfloat16
E4 = ml_dtypes.float8_e4m3


def build_program():
    nc = bass.Bass()
    # plane-major fp8 x: row index = (plane*NKD + kd)*128 + p, plane0=lo/1=hi
    xq = nc.declare_dram_parameter("xq", [2 * NKD * P, T], FP8, isOutput=False)
    # w1: e-block-major, per e-block columns = (plane, kd, q), plane0=hi/1=lo
    w1 = nc.declare_dram_parameter("w1", [NEB * P, 2 * NKD * P], FP8,
                                   isOutput=False)
    b1 = nc.declare_dram_parameter("b1", [P, NEB], F32, isOutput=False)
    # w2: row index = (kh*2 + plane)*128 + p, plane0=hi/1=lo
    w2 = nc.declare_dram_parameter("w2", [HC * 2 * P, D], FP8, isOutput=False)
    b2 = nc.declare_dram_parameter("b2", [P, D // P], F32, isOutput=False)
    maskp = nc.declare_dram_parameter("mask", [P, P], F16, isOutput=False)
    yt = nc.declare_dram_parameter("yt", [D, T], BF16, isOutput=True)

    with tile.TileContext(nc) as tc:
        with ExitStack() as ctx:
            _body(ctx, tc, nc, xq, w1, b1, w2, b2, maskp, yt)
    _legalize_waits(nc)
    return nc


def _legalize_waits(nc, nop_cap=1):
    """walrus's per-instruction sync-wait budget is tiny for matmuls (LDW+MM
    lowering) and DMA pseudo-instructions. Drop redundant same-engine
    self-waits (engines execute in order), then spill excess waits onto
    same-engine NoOps inserted right before the instruction."""
    nocap = (mybir.InstNoOp,)
    f = nc.m.functions[0]
    for bb in f.blocks:
        insts = bb.instructions
        for i in insts:
            si = i.sync_info
            if si is None or not si.on_wait:
                continue
            ename = str(i.engine).split(".")[-1]
            if ename == "SP":
                ename = "Sync"
            kept = [w for w in si.on_wait
                    if w.sync_type != "semaphore"
                    or w.wait_reg is not None
                    or not w.ant_name.split("_")[0] == ename]
            if len(kept) != len(si.on_wait):
                si.on_wait = kept
        idx = 0
        while idx < len(insts):
            i = insts[idx]
            si = i.sync_info
            cap = None if isinstance(i, nocap) else 1
            if cap is not None and si is not None and len(si.on_wait) > cap:
                excess = list(si.on_wait[:-cap])
                si.on_wait = list(si.on_wait[-cap:])
                while excess:
                    chunk, excess = excess[:nop_cap], excess[nop_cap:]
                    nop = mybir.InstNoOp(
                        name=nc.get_next_instruction_name(), ins=[], outs=[])
                    nop.engine = i.engine
                    nop.sync_info = mybir.SyncInfo(on_wait=chunk, on_update=[])
                    nc.register_instruction(nop)
                    insts.insert(idx, nop)
                    idx += 1
            idx += 1


class _CProj:
    """Stepwise emitter for one q-block's c_proj, interleaved into the NEXT
    q-block's (exp-paced) attention. Per me-tile: 6 DoubleRow matmuls
    (2 hi@hi kh-pairs + 4 cross per-kh); eviction with fused 2^-16 dequant
    and bias alternates DVE / ACT to split the f32-psum read load."""

    LAG = 1

    def __init__(self, nc, tb, ati, w2_sb, b2_sb, yt3, ps_acc, y_pool,
                 final=False):
        self.nc = nc
        self.tb = tb
        self.final = final
        if final:
            self.LAG = 2
        self.ati = ati
        self.w2_sb = w2_sb
        self.b2_sb = b2_sb
        self.yt3 = yt3
        self.ps_acc = ps_acc
        self.y_pool = y_pool
        self.ps_ys = {}
        self.y_t = None
        self.done = 0
        self.hdone = 0
        self.total = D // P + self.LAG

    def step(self):
        return self.half_step() and (self.half_step() or True)

    def half_step(self):
        if self.hdone >= 2 * self.total:
            return False
        me, phase = self.hdone // 2, self.hdone % 2
        self.hdone += 1
        self.done = self.hdone // 2
        nc = self.nc
        NME = D // P
        MG = NME // 8
        if phase == 0 and me >= self.LAG:
            md = me - self.LAG
            ps_y = self.ps_ys.pop(md)
            nc.tensor.matmul(ps_y[:],
                             self.w2_sb[:, HC - 1, :, md * P:(md + 1) * P],
                             self.ati[:, HC - 1, :, :],
                             start=False, stop=True, perf_mode=DR)
            mg, mi = md // MG, md % MG
            if mi == 0:
                y_t = self.y_pool.tile([P, MG, QT], BF16, tag="y")
                self.y_t = y_t
            if md % 4 != 1:
                nc.vector.tensor_scalar(self.y_t[:, mi, :], ps_y[:],
                                        DEQ, self.b2_sb[:, md:md + 1],
                                        ALU.mult, ALU.add)
            else:
                nc.scalar.activation(self.y_t[:, mi, :], ps_y[:],
                                     ACTF.Identity, scale=DEQ,
                                     bias=self.b2_sb[:, md:md + 1])
            if self.final and mg == NME // MG - 1:
                nc.sync.dma_start(
                    out=self.yt3[:, mg * MG + mi:mg * MG + mi + 1,
                                 self.tb:self.tb + QT],
                    in_=self.y_t[:, mi:mi + 1, :])
            elif mi == MG - 1:
                nc.sync.dma_start(
                    out=self.yt3[:, mg * MG:(mg + 1) * MG,
                                 self.tb:self.tb + QT],
                    in_=self.y_t[:])
        if phase == 1 and me < NME:
            ps_y = self.ps_acc.tile([P, QT], F32, tag="acc")
            self.ps_ys[me] = ps_y
            cols = slice(me * P, (me + 1) * P)
            nc.tensor.matmul(ps_y[:], self.w2_sb[:, 0:2, 0, cols],
                             self.ati[:, 0:2, 1, :],
                             start=True, stop=False, perf_mode=DR)
            nc.tensor.matmul(ps_y[:], self.w2_sb[:, 2:4, 0, cols],
                             self.ati[:, 2:4, 1, :],
                             start=False, stop=False, perf_mode=DR)
            for kh in range(HC - 1):
                nc.tensor.matmul(ps_y[:], self.w2_sb[:, kh, :, cols],
                                 self.ati[:, kh, :, :],
                                 start=False, stop=False, perf_mode=DR)
        return True


def _body(ctx, tc, nc, xq, w1, b1, w2, b2, maskp, yt):
    xq4 = xq.rearrange("(two kd p) t -> p two kd t", p=P, two=2)
    w13 = w1.rearrange("(eb p) x -> p eb x", p=P)
    w24 = w2.rearrange("(kh two p) d -> p kh two d", p=P, two=2)
    yt3 = yt.rearrange("(me p) t -> p me t", p=P)

    persist = ctx.enter_context(tc.tile_pool(name="persist", bufs=1))
    w1_sb = persist.tile([P, NEB, 2, NKD, P], FP8)   # [d_in, eb, hi/lo, kd, q]
    w2_sb = persist.tile([P, HC, 2, D], FP8)         # [dqc, kh, hi/lo, d_out]
    kt_sb = persist.tile([P, T], FP8)                # 16*K^T [dh, t] plain fp8
    v16_sb = persist.tile([P, T // P, DH], F16)      # 16*V [t_part, mt, dh]
    vhi_sb = persist.tile([P, T // P, DH], FP8)
    vlo_sb = persist.tile([P, T // P, DH], FP8)
    b1_sb = persist.tile([P, NEB], F32)
    b2_sb = persist.tile([P, D // P], F32)
    mask16 = persist.tile([P, P], F16)   # causal mask^T (0/-65504), PE-applied
    nbias = persist.tile([P, 1], F32)                # exp bias -2
    ones_mat = persist.tile([P, P], F16)             # 0.5: folds scales
    ident = persist.tile([P, P], F16)
    nc.vector.memset(ones_mat[:], 0.5)
    nc.vector.memset(nbias[:], EXPB)

    # w1 (per e-block) and the first q-block's xq are queued in the order
    # the first QKV e-block consumes them.
    xt_pool = ctx.enter_context(tc.tile_pool(name="xt", bufs=4))
    xt_first = []
    for _half in range(2):
        xt_c = xt_pool.tile([P, 2, NKC, QT], FP8, tag="xt")
        xt_first.append(xt_c)
    W1C = 2 * NKD * P
    # (kind, eb/chunk, plane): eb0's hi weights + both chunks' hi planes
    # first, so the 16 hi@hi matmuls of eb0 start after ~1.5MB of DMA.
    for kind, a, pl in [
            ('w1', 0, 0), ('xt', 0, 1), ('xt', 1, 1), ('w1', 0, 1),
            ('xt', 0, 0), ('xt', 1, 0), ('w1', 1, 0), ('w1', 1, 1),
            ('w1', 2, None), ('w1', 3, None), ('w1', 4, None),
            ('w1', 5, None)]:
        if kind == 'w1':
            if pl is None:
                nc.sync.dma_start(
                    out=w1_sb[:, a].rearrange("p two kd q -> p (two kd q)"),
                    in_=w13[:, a, :])
            else:
                nc.sync.dma_start(
                    out=w1_sb[:, a, pl], in_=w13[:, a, pl * (W1C // 2):
                                                 (pl + 1) * (W1C // 2)]
                    .rearrange("p (kd q) -> p kd q", q=P))
        else:
            nc.sync.dma_start(
                out=xt_first[a][:, pl, :, :],
                in_=xq4[:, pl, a * NKC:(a + 1) * NKC, 0:QT])
    nc.sync.dma_start(out=b1_sb[:], in_=b1[:])
    nc.sync.dma_start(out=mask16[:], in_=maskp[:])
    make_identity(nc, ident[:])
    nc.sync.dma_start(out=w2_sb[:], in_=w24[:])
    nc.sync.dma_start(out=b2_sb[:], in_=b2[:])

    # PSUM: 3 (acc) + 2*2 (score pairs) + 1 (misc) = 8 banks
    ps_acc = ctx.enter_context(tc.tile_pool(name="ps_acc", bufs=3, space="PSUM"))
    ps_pair = ctx.enter_context(tc.tile_pool(name="ps_pair", bufs=2, space="PSUM"))
    ps_misc = ctx.enter_context(tc.tile_pool(name="ps_misc", bufs=1, space="PSUM"))

    qt_pool = ctx.enter_context(tc.tile_pool(name="qt", bufs=2))
    q16_pool = ctx.enter_context(tc.tile_pool(name="q16", bufs=2))
    vs_pool = ctx.enter_context(tc.tile_pool(name="vs", bufs=2))
    p_pool = ctx.enter_context(tc.tile_pool(name="pp", bufs=3))
    p16_pool = ctx.enter_context(tc.tile_pool(name="p16", bufs=3))
    tm_pool = ctx.enter_context(tc.tile_pool(name="tm", bufs=2))
    psum_pool = ctx.enter_context(tc.tile_pool(name="psm", bufs=2))
    ibc_pool = ctx.enter_context(tc.tile_pool(name="ibc", bufs=2))
    at16_pool = ctx.enter_context(tc.tile_pool(name="a16", bufs=2))
    ati_pool = ctx.enter_context(tc.tile_pool(name="ati", bufs=2))
    y_pool = ctx.enter_context(tc.tile_pool(name="yp", bufs=2))

    class _QKV:
        """Stepwise emitter for one q-block's QKV: per e-block, 16 hi@hi
        DoubleRow (adjacent kd pairs, hi planes) + 32 cross DoubleRow
        (per-kd (w_hi,x_lo)+(w_lo,x_hi)) into one PSUM group."""

        def __init__(self, j, xt_cs):
            self.tb = j * QT
            self.xt_cs = xt_cs
            self.qi = qt_pool.tile([P, HC, 2, QT], FP8, tag="qt")
            self.v_st = None
            self.eb = 0
            self.mi = 0
            self.ps = None
            self.total_mm = NEB * NQKV
            self.done_mm = 0

        def step(self, n_mm=8):
            if self.eb >= NEB:
                return False
            for _ in range(n_mm):
                if self.ps is None:
                    self.ps = ps_acc.tile([P, QT], F32, tag="acc")
                eb, mi = self.eb, self.mi
                if mi < NKD // 2:          # hi@hi: kd pair (2mi, 2mi+1)
                    kd0 = 2 * mi
                    c, r = kd0 // NKC, kd0 % NKC
                    nc.tensor.matmul(
                        self.ps[:], w1_sb[:, eb, 0, kd0:kd0 + 2, :],
                        self.xt_cs[c][:, 1, r:r + 2, :],
                        start=(mi == 0), stop=False, perf_mode=DR)
                else:                      # cross: kd = mi - 16
                    kd = mi - NKD // 2
                    c, r = kd // NKC, kd % NKC
                    nc.tensor.matmul(
                        self.ps[:], w1_sb[:, eb, :, kd, :],
                        self.xt_cs[c][:, :, r, :],
                        start=False, stop=(mi == NQKV - 1), perf_mode=DR)
                self.done_mm += 1
                self.mi += 1
                if self.mi == NQKV:
                    self._evict()
                    self.mi = 0
                    self.eb += 1
                    self.ps = None
                    if self.eb >= NEB:
                        return False
            return True

        def _evict(self):
            eb, ps = self.eb, self.ps
            # b1 is pre-scaled x16 on host for all columns
            if eb < HC:      # Q head: 16*q -> f16 master, then fp8 hi+lo
                q16 = q16_pool.tile([P, QT], F16, tag="q16")
                nc.scalar.activation(q16[:], ps[:],
                                     ACTF.Identity, scale=DEQ * SV,
                                     bias=b1_sb[:, eb:eb + 1])
                nc.scalar.copy(self.qi[:, eb, 1, :], q16[:])
                nc.gpsimd.tensor_sub(self.qi[:, eb, 0, :], q16[:],
                                     self.qi[:, eb, 1, :])
            elif eb == HC:   # K^T: plain fp8 x16
                nc.scalar.activation(kt_sb[:, self.tb:self.tb + QT], ps[:],
                                     ACTF.Identity, scale=DEQ * SV,
                                     bias=b1_sb[:, eb:eb + 1])
            else:            # V: 16*(v+b) -> f16 on DVE
                v_s = vs_pool.tile([P, QT], F16, tag="vs")
                nc.vector.tensor_scalar(v_s[:], ps[:], DEQ * SV,
                                        b1_sb[:, eb:eb + 1],
                                        ALU.mult, ALU.add)
                self.v_st = v_s

    def prefetch_xq(jp):
        cs = []
        for half in range(2):
            xt_c = xt_pool.tile([P, 2, NKC, QT], FP8, tag="xt")
            for pl in (1, 0):
                nc.sync.dma_start(
                    out=xt_c[:, pl, :, :],
                    in_=xq4[:, pl, half * NKC:(half + 1) * NKC,
                             jp * QT:jp * QT + QT])
            cs.append(xt_c)
        return cs

    cproj_prev = None
    qkv_cur = None
    qkv_next = None
    xt_next = None
    for j in range(NJ):
        b, jj = j // NJB, j % NJB
        tb = j * QT

        # ---- QKV for tokens [tb, tb+QT) -----------------------------------
        qkv_cur = qkv_next if qkv_next is not None else _QKV(j, xt_first)
        qkv_next = None
        if j + 1 < NJ and j > 0:
            xt_next = prefetch_xq(j + 1)
        while qkv_cur.step():
            pass
        qi = qkv_cur.qi
        v_st = qkv_cur.v_st

        # ---- attention for this q-block (4 heads) -------------------------
        # Scores: one DoubleRow per k-tile (K^T stride-0-duplicated in the
        # stationary slots, q hi+lo in the moving slots). Off-diag pairs:
        # one fp8 exp covers both k-tiles, then 2 DoubleRow PV (vhi, vlo
        # slot-paired across the pair). Diagonal: fp16 probs and fp16 V.
        # Denominator: per-pair GPSIMD combine (fp8+fp8->f16) + fp16-only
        # DVE accumulate chain (2x mode); one 0.5-matmul per head reduces
        # and broadcasts it with the x16 V / x32 at scales folded in.
        ati = ati_pool.tile([P, HC, 2, QT], FP8, tag="ati")  # plane0=lo/1=hi
        nk = 4 * jj + 4
        units = [(kk, kk + 1) for kk in range(0, 4 * jj, 2)] \
            + [(kk,) for kk in range(4 * jj, nk)]

        def emit_unit(h, u):
            kks = units[u]
            psp = ps_pair.tile([P, 2, QT], F32, tag="pair")
            if len(kks) == 2:
                p8 = p_pool.tile([P, 2, QT], FP8, tag="p")
                for i, kk in enumerate(kks):
                    c0 = b * S + kk * P
                    k_dup = (kt_sb[:, c0:c0 + P]
                             .rearrange("p (one q) -> p one q", one=1)
                             .broadcast_to([P, 2, P]))
                    nc.tensor.matmul(psp[:, i, :], k_dup, qi[:, h, :, :],
                                     start=True, stop=True, perf_mode=DR)
                nc.scalar.activation(p8[:, :, :], psp[:, :, :],
                                     ACTF.Exp, scale=SCALE / (SV * SV),
                                     bias=nbias[:])
                tm = tm_pool.tile([P, QT], F16, tag="tm")
                nc.gpsimd.tensor_add(tm[:], p8[:, 0, :], p8[:, 1, :])
                return ('off', p8, kks[0], tm)
            kk = kks[0]
            qoff = P * (kk - 4 * jj)
            p16 = p16_pool.tile([P, QT], F16, tag="p16")
            c0 = b * S + kk * P
            k_dup = (kt_sb[:, c0:c0 + P]
                     .rearrange("p (one q) -> p one q", one=1)
                     .broadcast_to([P, 2, P]))
            nc.tensor.matmul(psp[:, 0, qoff:], k_dup, qi[:, h, :, qoff:],
                             start=True, stop=False, perf_mode=DR)
            # causal mask for the diagonal 128x128: one f16 matmul
            # (mask^T stationary x identity) accumulates 0/-65504 into the
            # scores -- keeps the mask off the DVE and out of its queue
            nc.tensor.matmul(psp[:, 0, qoff:qoff + P], mask16[:], ident[:],
                             start=False, stop=True)
            nc.scalar.activation(p16[:, qoff:], psp[:, 0, qoff:],
                                 ACTF.Exp, scale=SCALE / (SV * SV),
                                 bias=nbias[:])
            return ('diag', p16, kk, qoff)

        def finalize_head(h, ps_out, p_sum):
            # 0.5-matmul: denominator broadcast across partitions with the
            # x16 V and /32 at scales folded in; then normalize and split
            # the c_proj input into fp8 hi+lo planes.
            ps_db = ps_misc.tile([P, QT], F32, tag="misc")
            nc.tensor.matmul(ps_db[:], ones_mat[:], p_sum[:],
                             start=True, stop=True)
            inv_bc = ibc_pool.tile([P, QT], F16, tag="ibc")
            with nc.allow_low_precision(reason="f16 inv: den spans 2e-3..500, "
                                        "1e-3 rel err ≪ fp8 prob noise"):
                nc.vector.reciprocal(inv_bc[:], ps_db[:])
            at16 = at16_pool.tile([P, QT], F16, tag="a16")
            nc.vector.tensor_mul(at16[:], ps_out[:], inv_bc[:])
            nc.scalar.copy(ati[:, h, 1, :], at16[:])
            nc.gpsimd.tensor_sub(ati[:, h, 0, :], at16[:], ati[:, h, 1, :])

        NU = len(units)
        stream = [(h, u) for h in range(HC) for u in range(NU)]
        total_units = len(stream)
        units_done = 0
        pending = None
        ps_out = None
        p_sum = None
        u_next = emit_unit(*stream[0])
        # V transposes (fp16) for this q-block, then fp8 hi/lo planes
        for i in range(QT // P):
            tp = ps_acc.tile([P, P], F16, tag="acc")
            nc.tensor.transpose(tp[:], v_st[:, i * P:(i + 1) * P],
                                ident[:])
            mt = j * (QT // P) + i
            nc.vector.tensor_copy(v16_sb[:, mt, :], tp[:])
            nc.scalar.copy(vhi_sb[:, mt, :], v16_sb[:, mt, :])
            nc.gpsimd.tensor_sub(vlo_sb[:, mt, :], v16_sb[:, mt, :],
                                 vhi_sb[:, mt, :])
        if j + 1 < NJ:
            if j == 0:
                xt_next = prefetch_xq(1)
            qkv_next = _QKV(j + 1, xt_next)
        for idx, (h, u) in enumerate(stream):
            kind, pt, kk0, extra = u_next
            if u == 0 and pending is not None:
                finalize_head(*pending)
                pending = None
            if idx + 1 < total_units:
                u_next = emit_unit(*stream[idx + 1])
            if u == 0:
                ps_out = ps_acc.tile([P, QT], F32, tag="acc")
                p_sum = psum_pool.tile([P, QT], F16, tag="psum")
            # filler BEFORE this unit's PV matmuls (cover the exp latency
            # the PV waits on): previous block's c_proj, then the next
            # block's QKV
            if cproj_prev is not None:
                target = 2 * cproj_prev.total * (units_done + 2) // total_units
                while cproj_prev.hdone < target and cproj_prev.half_step():
                    pass
            if qkv_next is not None and units_done > 0:
                target = qkv_next.total_mm * (units_done + 2) // total_units
                while qkv_next.done_mm < target and qkv_next.step(8):
                    pass
            if kind == 'off':
                mt0 = b * (S // P) + kk0
                nc.tensor.matmul(ps_out[:], vhi_sb[:, mt0:mt0 + 2, :],
                                 pt[:, :, :], start=(kk0 == 0), stop=False,
                                 perf_mode=DR)
                nc.tensor.matmul(ps_out[:], vlo_sb[:, mt0:mt0 + 2, :],
                                 pt[:, :, :], start=False, stop=False,
                                 perf_mode=DR)
                if kk0 == 0:
                    nc.vector.tensor_copy(p_sum[:], extra[:])
                else:
                    nc.vector.tensor_add(p_sum[:], p_sum[:], extra[:])
            else:
                kk, qoff = kk0, extra
                nc.tensor.matmul(ps_out[:, qoff:],
                                 v16_sb[:, b * (S // P) + kk, :],
                                 pt[:, qoff:], start=(kk == 0),
                                 stop=(kk == nk - 1))
                if kk == 0:
                    nc.vector.tensor_copy(p_sum[:], pt[:])
                else:
                    nc.vector.tensor_add(p_sum[:, qoff:], p_sum[:, qoff:],
                                         pt[:, qoff:])
            units_done += 1
            if u == NU - 1:
                pending = (h, ps_out, p_sum)
        finalize_head(*pending)
        if cproj_prev is not None:
            while cproj_prev.step():
                pass
        cproj_prev = _CProj(nc, tb, ati, w2_sb, b2_sb, yt3,
                            ps_acc, y_pool, final=(j == NJ - 1))
    while cproj_prev.step():
        pass


_PROGRAM = None


def _get_program():
    global _PROGRAM
    if _PROGRAM is None:
        _PROGRAM = build_program()
    return _PROGRAM


def _split8(a):
    hi = a.astype(E4)
    lo = (a - hi.astype(np.float32)).astype(E4)
    return hi, lo


def make_in_maps(hidden_states, w_qkv, b_qkv, w_proj, b_proj):
    x = np.asarray(hidden_states, dtype=np.float32).reshape(T, D)
    xs = np.ascontiguousarray(x.T) * SX          # [D, T]
    xhi, xlo = _split8(xs)
    xhi_r = xhi.reshape(NKD, P, T)
    xlo_r = xlo.reshape(NKD, P, T)
    xq = np.ascontiguousarray(
        np.concatenate([xlo_r, xhi_r], axis=0).reshape(2 * NKD * P, T))
    pi = np.arange(P)[:, None]
    kk = np.arange(P)[None, :]
    mask = np.where(kk <= pi, 0.0, -65504.0).astype(np.float16)
    mask = np.ascontiguousarray(mask)
    w_qkv = np.asarray(w_qkv, dtype=np.float32)
    b_qkv = np.asarray(b_qkv, dtype=np.float32)
    w_proj = np.asarray(w_proj, dtype=np.float32)
    b_proj = np.asarray(b_proj, dtype=np.float32)
    b2 = np.ascontiguousarray(
        (b_proj / NCORES).reshape(D // P, P).T).astype(np.float32)
    in_maps = []
    for c in range(NCORES):
        qcols = slice(c * DQC, (c + 1) * DQC)
        wsel = np.concatenate([w_qkv[:, qcols], w_qkv[:, D:]], axis=1) * SW
        whi, wlo = _split8(wsel)                  # [D, E1]
        # -> [eb, p, plane, kd, q]; plane0=hi
        w1 = np.stack([whi.reshape(NKD, P, NEB, P),
                       wlo.reshape(NKD, P, NEB, P)], axis=0)
        w1 = w1.transpose(3, 2, 0, 1, 4).reshape(NEB * P, 2 * NKD * P)
        b1 = SV * np.concatenate([b_qkv[qcols], b_qkv[D:]])
        wps = w_proj[c * DQC:(c + 1) * DQC, :] * SW
        w2hi, w2lo = _split8(wps)                 # [DQC, D]
        w2 = np.stack([w2hi.reshape(HC, P, D),
                       w2lo.reshape(HC, P, D)], axis=1).reshape(HC * 2 * P, D)
        in_maps.append({
            "xq": xq,
            "w1": np.ascontiguousarray(w1),
            "b1": np.ascontiguousarray(b1.reshape(NEB, P).T).astype(np.float32),
            "w2": np.ascontiguousarray(w2),
            "b2": b2,
            "mask": mask,
        })
    return in_maps


def kernel(hidden_states, w_qkv, b_qkv, w_proj, b_proj):
    nc = _get_program()
    in_maps = make_in_maps(hidden_states, w_qkv, b_qkv, w_proj, b_proj)
    res = run_bass_kernel_spmd(nc, in_maps, list(range(NCORES)))
    y = np.zeros((D, T), dtype=np.float32)
    for r in res.results:
        y += np.asarray(r["yt"]).astype(np.float32)
    return np.ascontiguousarray(y.T.reshape(B, S, D))


# revision 23
# speedup vs baseline: 1.0364x; 1.0364x over previous
"""GPTBigCode MQA causal attention block on 8 TRN2 NeuronCores — v4.

Tensor-parallel over heads (4 of 32 query heads per core, single KV head
replicated), row-parallel c_proj, bf16 partial outputs summed on host.

v4 = v3 (fp8 DoubleRow GEMMs + fp8 off-diag attention) plus:
- Scores in fp8 DoubleRow at 0.5x: K^T is plain fp8 (x16) duplicated
  across both stationary slots via a stride-0 AP; Q is split hi+lo fp8
  (x16) in the two moving slots, so one DoubleRow computes
  k8^T(q_hi+q_lo) — full Q precision, only K carries plain-fp8 error.
- Engine rebalance (GPSIMD cannot touch PSUM; DVE's 2x path needs all
  operands 2-byte): y evictions alternate DVE/ACT; SBUF-only fp8 ops
  (prob pair combine, q/v/at lo-splits) go to the idle GPSIMD; the
  softmax-denominator accumulation chain is fp16-only on DVE (2x mode)
  fed by per-pair GPSIMD combines that run in parallel.
"""

import numpy as np
from contextlib import ExitStack

import ml_dtypes
import concourse.bass as bass
import concourse.tile as tile
from concourse import bass_isa, mybir
from concourse.bass_utils import run_bass_kernel_spmd
from concourse.masks import make_identity

B, S, D = 2, 2048, 4096
H, DH = 32, 128
NCORES = 8
HC = H // NCORES          # 4 heads per core
DQC = HC * DH             # 512 q-dims per core
T = B * S                 # 4096 tokens
P = 128
NKD = D // P              # 32 contraction tiles in model dim
E1 = DQC + 2 * DH         # 768 per-core QKV output dims
NEB = E1 // P             # 6 e-blocks: 4 Q heads, K, V
QT = 512                  # tokens per (b,j) group
NJ = T // QT              # 8 groups
NJB = S // QT             # 4 groups per batch
SCALE = DH ** -0.5
NKC = NKD // 2            # kd tiles per xq chunk

SX = 32.0                 # x fp8 scale
SW = 2048.0               # weight fp8 scale
SV = 16.0                 # q/k/v scale (fp16 master + fp8)
DEQ = 1.0 / (SX * SW)     # 2^-16
EXPB = -2.0               # exp bias: p~ = e^(s*SCALE - 2)
NQKV = 48                 # DoubleRow instrs per QKV e-block

F32 = mybir.dt.float32
R32 = mybir.dt.float32r
BF16 = mybir.dt.bfloat16
F16 = mybir.dt.float16
FP8 = mybir.dt.float8e4
ACTF = mybir.ActivationFunctionType
DR = mybir.MatmulPerfMode.DoubleRow
ALU = mybir.AluOpType
NEG = -1.0e30
BF = ml_dtypes.bfloat16
E4 = ml_dtypes.float8_e4m3


def build_program():
    nc = bass.Bass()
    # plane-major fp8 x: row index = (plane*NKD + kd)*128 + p, plane0=lo/1=hi
    xq = nc.declare_dram_parameter("xq", [2 * NKD * P, T], FP8, isOutput=False)
    # w1: e-block-major, per e-block columns = (plane, kd, q), plane0=hi/1=lo
    w1 = nc.declare_dram_parameter("w1", [NEB * P, 2 * NKD * P], FP8,
                                   isOutput=False)
    b1 = nc.declare_dram_parameter("b1", [P, NEB], F32, isOutput=False)
    # w2: row index = (kh*2 + plane)*128 + p, plane0=hi/1=lo
    w2 = nc.declare_dram_parameter("w2", [HC * 2 * P, D], FP8, isOutput=False)
    b2 = nc.declare_dram_parameter("b2", [P, D // P], F32, isOutput=False)
    maskp = nc.declare_dram_parameter("mask", [P, P], F16, isOutput=False)
    yt = nc.declare_dram_parameter("yt", [D, T], BF16, isOutput=True)

    with tile.TileContext(nc) as tc:
        with ExitStack() as ctx:
            _body(ctx, tc, nc, xq, w1, b1, w2, b2, maskp, yt)
    _legalize_waits(nc)
    return nc


def _legalize_waits(nc, nop_cap=1):
    """walrus's per-instruction sync-wait budget is tiny for matmuls (LDW+MM
    lowering) and DMA pseudo-instructions. Drop redundant same-engine
    self-waits (engines execute in order), then spill excess waits onto
    same-engine NoOps inserted right before the instruction."""
    nocap = (mybir.InstNoOp,)
    f = nc.m.functions[0]
    for bb in f.blocks:
        insts = bb.instructions
        for i in insts:
            si = i.sync_info
            if si is None or not si.on_wait:
                continue
            ename = str(i.engine).split(".")[-1]
            if ename == "SP":
                ename = "Sync"
            kept = [w for w in si.on_wait
                    if w.sync_type != "semaphore"
                    or w.wait_reg is not None
                    or not w.ant_name.split("_")[0] == ename]
            if len(kept) != len(si.on_wait):
                si.on_wait = kept
        idx = 0
        while idx < len(insts):
            i = insts[idx]
            si = i.sync_info
            cap = None if isinstance(i, nocap) else 1
            if cap is not None and si is not None and len(si.on_wait) > cap:
                excess = list(si.on_wait[:-cap])
                si.on_wait = list(si.on_wait[-cap:])
                while excess:
                    chunk, excess = excess[:nop_cap], excess[nop_cap:]
                    nop = mybir.InstNoOp(
                        name=nc.get_next_instruction_name(), ins=[], outs=[])
                    nop.engine = i.engine
                    nop.sync_info = mybir.SyncInfo(on_wait=chunk, on_update=[])
                    nc.register_instruction(nop)
                    insts.insert(idx, nop)
                    idx += 1
            idx += 1


class _CProj:
    """Stepwise emitter for one q-block's c_proj, interleaved into the NEXT
    q-block's (exp-paced) attention. Per me-tile: 6 DoubleRow matmuls
    (2 hi@hi kh-pairs + 4 cross per-kh); eviction with fused 2^-16 dequant
    and bias alternates DVE / ACT to split the f32-psum read load."""

    LAG = 1

    def __init__(self, nc, tb, ati, w2_sb, b2_sb, yt3, ps_acc, y_pool,
                 final=False):
        self.nc = nc
        self.tb = tb
        self.final = final
        if final:
            self.LAG = 2
        self.ati = ati
        self.w2_sb = w2_sb
        self.b2_sb = b2_sb
        self.yt3 = yt3
        self.ps_acc = ps_acc
        self.y_pool = y_pool
        self.ps_ys = {}
        self.y_t = None
        self.done = 0
        self.hdone = 0
        self.total = D // P + self.LAG

    def step(self):
        return self.half_step() and (self.half_step() or True)

    def half_step(self):
        if self.hdone >= 2 * self.total:
            return False
        me, phase = self.hdone // 2, self.hdone % 2
        self.hdone += 1
        self.done = self.hdone // 2
        nc = self.nc
        NME = D // P
        MG = NME // 8
        if phase == 0 and me >= self.LAG:
            md = me - self.LAG
            ps_y = self.ps_ys.pop(md)
            nc.tensor.matmul(ps_y[:],
                             self.w2_sb[:, HC - 1, :, md * P:(md + 1) * P],
                             self.ati[:, HC - 1, :, :],
                             start=False, stop=True, perf_mode=DR)
            mg, mi = md // MG, md % MG
            if mi == 0:
                y_t = self.y_pool.tile([P, MG, QT], BF16, tag="y")
                self.y_t = y_t
            if True:
                nc.vector.tensor_scalar(self.y_t[:, mi, :], ps_y[:],
                                        DEQ, self.b2_sb[:, md:md + 1],
                                        ALU.mult, ALU.add)
            else:
                nc.scalar.activation(self.y_t[:, mi, :], ps_y[:],
                                     ACTF.Identity, scale=DEQ,
                                     bias=self.b2_sb[:, md:md + 1])
            if self.final and mg == NME // MG - 1:
                nc.sync.dma_start(
                    out=self.yt3[:, mg * MG + mi:mg * MG + mi + 1,
                                 self.tb:self.tb + QT],
                    in_=self.y_t[:, mi:mi + 1, :])
            elif mi == MG - 1:
                nc.sync.dma_start(
                    out=self.yt3[:, mg * MG:(mg + 1) * MG,
                                 self.tb:self.tb + QT],
                    in_=self.y_t[:])
        if phase == 1 and me < NME:
            ps_y = self.ps_acc.tile([P, QT], F32, tag="acc")
            self.ps_ys[me] = ps_y
            cols = slice(me * P, (me + 1) * P)
            nc.tensor.matmul(ps_y[:], self.w2_sb[:, 0:2, 0, cols],
                             self.ati[:, 0:2, 1, :],
                             start=True, stop=False, perf_mode=DR)
            nc.tensor.matmul(ps_y[:], self.w2_sb[:, 2:4, 0, cols],
                             self.ati[:, 2:4, 1, :],
                             start=False, stop=False, perf_mode=DR)
            for kh in range(HC - 1):
                nc.tensor.matmul(ps_y[:], self.w2_sb[:, kh, :, cols],
                                 self.ati[:, kh, :, :],
                                 start=False, stop=False, perf_mode=DR)
        return True


def _body(ctx, tc, nc, xq, w1, b1, w2, b2, maskp, yt):
    xq4 = xq.rearrange("(two kd p) t -> p two kd t", p=P, two=2)
    w13 = w1.rearrange("(eb p) x -> p eb x", p=P)
    w24 = w2.rearrange("(kh two p) d -> p kh two d", p=P, two=2)
    yt3 = yt.rearrange("(me p) t -> p me t", p=P)

    persist = ctx.enter_context(tc.tile_pool(name="persist", bufs=1))
    w1_sb = persist.tile([P, NEB, 2, NKD, P], FP8)   # [d_in, eb, hi/lo, kd, q]
    w2_sb = persist.tile([P, HC, 2, D], FP8)         # [dqc, kh, hi/lo, d_out]
    kt_sb = persist.tile([P, T], FP8)                # 16*K^T [dh, t] plain fp8
    v16_sb = persist.tile([P, T // P, DH], F16)      # 16*V [t_part, mt, dh]
    vhi_sb = persist.tile([P, T // P, DH], FP8)
    vlo_sb = persist.tile([P, T // P, DH], FP8)
    b1_sb = persist.tile([P, NEB], F32)
    b2_sb = persist.tile([P, D // P], F32)
    mask16 = persist.tile([P, P], F16)   # causal mask^T (0/-65504), PE-applied
    nbias = persist.tile([P, 1], F32)                # exp bias -2
    ones_mat = persist.tile([P, P], F16)             # 0.5: folds scales
    ones8 = persist.tile([P, P], FP8)                # 0.5 for fp8 den DR
    ident = persist.tile([P, P], F16)
    nc.vector.memset(ones_mat[:], 0.5)
    nc.vector.memset(ones8[:], 0.5)
    nc.vector.memset(nbias[:], EXPB)

    # w1 (per e-block) and the first q-block's xq are queued in the order
    # the first QKV e-block consumes them.
    xt_pool = ctx.enter_context(tc.tile_pool(name="xt", bufs=3))
    xt_first = []
    for _half in range(2):
        xt_c = xt_pool.tile([P, 2, NKC, QT], FP8, tag="xt")
        xt_first.append(xt_c)
    W1C = 2 * NKD * P
    # (kind, eb/chunk, plane): eb0's hi weights + both chunks' hi planes
    # first, so the 16 hi@hi matmuls of eb0 start after ~1.5MB of DMA.
    for kind, a, pl in [
            ('w1', 0, 0), ('xt', 0, 1), ('xt', 1, 1), ('w1', 0, 1),
            ('xt', 0, 0), ('xt', 1, 0), ('w1', 1, 0), ('w1', 1, 1),
            ('w1', 2, None), ('w1', 3, None), ('w1', 4, None),
            ('w1', 5, None)]:
        if kind == 'w1':
            if pl is None:
                nc.sync.dma_start(
                    out=w1_sb[:, a].rearrange("p two kd q -> p (two kd q)"),
                    in_=w13[:, a, :])
            else:
                nc.sync.dma_start(
                    out=w1_sb[:, a, pl], in_=w13[:, a, pl * (W1C // 2):
                                                 (pl + 1) * (W1C // 2)]
                    .rearrange("p (kd q) -> p kd q", q=P))
        else:
            nc.scalar.dma_start(
                out=xt_first[a][:, pl, :, :],
                in_=xq4[:, pl, a * NKC:(a + 1) * NKC, 0:QT])
    nc.scalar.dma_start(out=b1_sb[:], in_=b1[:])
    nc.scalar.dma_start(out=mask16[:], in_=maskp[:])
    make_identity(nc, ident[:])

    # PSUM: 3 (acc) + 2*2 (score pairs) + 1 (misc) = 8 banks
    ps_acc = ctx.enter_context(tc.tile_pool(name="ps_acc", bufs=3, space="PSUM"))
    ps_pair = ctx.enter_context(tc.tile_pool(name="ps_pair", bufs=2, space="PSUM"))
    ps_misc = ctx.enter_context(tc.tile_pool(name="ps_misc", bufs=1, space="PSUM"))

    qt_pool = ctx.enter_context(tc.tile_pool(name="qt", bufs=2))
    q16_pool = ctx.enter_context(tc.tile_pool(name="q16", bufs=2))
    vs_pool = ctx.enter_context(tc.tile_pool(name="vs", bufs=2))
    p_pool = ctx.enter_context(tc.tile_pool(name="pp", bufs=3))
    p16_pool = ctx.enter_context(tc.tile_pool(name="p16", bufs=3))
    psum_pool = ctx.enter_context(tc.tile_pool(name="psm", bufs=2))
    ibc_pool = ctx.enter_context(tc.tile_pool(name="ibc", bufs=2))
    at16_pool = ctx.enter_context(tc.tile_pool(name="a16", bufs=2))
    ati_pool = ctx.enter_context(tc.tile_pool(name="ati", bufs=3))
    y_pool = ctx.enter_context(tc.tile_pool(name="yp", bufs=2))

    class _QKV:
        """Stepwise emitter for one q-block's QKV: per e-block, 16 hi@hi
        DoubleRow (adjacent kd pairs, hi planes) + 32 cross DoubleRow
        (per-kd (w_hi,x_lo)+(w_lo,x_hi)) into one PSUM group."""

        def __init__(self, j, xt_cs):
            self.tb = j * QT
            self.xt_cs = xt_cs
            self.qi = qt_pool.tile([P, HC, 2, QT], FP8, tag="qt")
            self.v_st = None
            self.eb = 0
            self.mi = 0
            self.ps = None
            self.total_mm = NEB * NQKV
            self.done_mm = 0

        def step(self, n_mm=8):
            if self.eb >= NEB:
                return False
            for _ in range(n_mm):
                if self.ps is None:
                    self.ps = ps_acc.tile([P, QT], F32, tag="acc")
                eb, mi = self.eb, self.mi
                if mi < NKD // 2:          # hi@hi: kd pair (2mi, 2mi+1)
                    kd0 = 2 * mi
                    c, r = kd0 // NKC, kd0 % NKC
                    nc.tensor.matmul(
                        self.ps[:], w1_sb[:, eb, 0, kd0:kd0 + 2, :],
                        self.xt_cs[c][:, 1, r:r + 2, :],
                        start=(mi == 0), stop=False, perf_mode=DR)
                else:                      # cross: kd = mi - 16
                    kd = mi - NKD // 2
                    c, r = kd // NKC, kd % NKC
                    nc.tensor.matmul(
                        self.ps[:], w1_sb[:, eb, :, kd, :],
                        self.xt_cs[c][:, :, r, :],
                        start=False, stop=(mi == NQKV - 1), perf_mode=DR)
                self.done_mm += 1
                self.mi += 1
                if self.mi == NQKV:
                    self._evict()
                    self.mi = 0
                    self.eb += 1
                    self.ps = None
                    if self.eb >= NEB:
                        return False
            return True

        def _evict(self):
            eb, ps = self.eb, self.ps
            # b1 is pre-scaled x16 on host for all columns
            if eb < HC:      # Q head: 16*q -> f16 master, then fp8 hi+lo
                q16 = q16_pool.tile([P, QT], F16, tag="q16")
                nc.scalar.activation(q16[:], ps[:],
                                     ACTF.Identity, scale=DEQ * SV,
                                     bias=b1_sb[:, eb:eb + 1])
                nc.scalar.copy(self.qi[:, eb, 1, :], q16[:])
                nc.vector.tensor_sub(self.qi[:, eb, 0, :], q16[:],
                                     self.qi[:, eb, 1, :])
            elif eb == HC:   # K^T: plain fp8 x16
                nc.scalar.activation(kt_sb[:, self.tb:self.tb + QT], ps[:],
                                     ACTF.Identity, scale=DEQ * SV,
                                     bias=b1_sb[:, eb:eb + 1])
            else:            # V: 16*(v+b) -> f16 on DVE
                v_s = vs_pool.tile([P, QT], F16, tag="vs")
                nc.vector.tensor_scalar(v_s[:], ps[:], DEQ * SV,
                                        b1_sb[:, eb:eb + 1],
                                        ALU.mult, ALU.add)
                self.v_st = v_s

    def prefetch_xq(jp):
        cs = []
        for half in range(2):
            xt_c = xt_pool.tile([P, 2, NKC, QT], FP8, tag="xt")
            for pl in (1, 0):
                nc.sync.dma_start(
                    out=xt_c[:, pl, :, :],
                    in_=xq4[:, pl, half * NKC:(half + 1) * NKC,
                             jp * QT:jp * QT + QT])
            cs.append(xt_c)
        return cs

    SPILL = 16     # half-steps of c_proj spilled into the next (jj=3) window
    cproj_q = []
    qkv_cur = None
    qkv_next = None
    xt_next = None
    for j in range(NJ):
        b, jj = j // NJB, j % NJB
        tb = j * QT

        # ---- QKV for tokens [tb, tb+QT) -----------------------------------
        qkv_cur = qkv_next if qkv_next is not None else _QKV(j, xt_first)
        qkv_next = None
        if j + 1 < NJ and j > 0:
            xt_next = prefetch_xq(j + 1)
        while qkv_cur.step():
            pass
        qi = qkv_cur.qi
        v_st = qkv_cur.v_st

        # ---- attention for this q-block (4 heads) -------------------------
        # Scores: one DoubleRow per k-tile (K^T stride-0-duplicated in the
        # stationary slots, q hi+lo in the moving slots). Off-diag pairs:
        # one fp8 exp covers both k-tiles, then 2 DoubleRow PV (vhi, vlo
        # slot-paired across the pair). Diagonal: fp16 probs and fp16 V.
        # Denominator: per-pair GPSIMD combine (fp8+fp8->f16) + fp16-only
        # DVE accumulate chain (2x mode); one 0.5-matmul per head reduces
        # and broadcasts it with the x16 V / x32 at scales folded in.
        ati = ati_pool.tile([P, HC, 2, QT], FP8, tag="ati")  # plane0=lo/1=hi
        nk = 4 * jj + 4
        units = [(kk, kk + 1) for kk in range(0, 4 * jj, 2)] \
            + [(kk,) for kk in range(4 * jj, nk)]

        def emit_unit(h, u):
            kks = units[u]
            psp = ps_pair.tile([P, 2, QT], F32, tag="pair")
            if len(kks) == 2:
                p8 = p_pool.tile([P, 2, QT], FP8, tag="p")
                for i, kk in enumerate(kks):
                    c0 = b * S + kk * P
                    k_dup = (kt_sb[:, c0:c0 + P]
                             .rearrange("p (one q) -> p one q", one=1)
                             .broadcast_to([P, 2, P]))
                    nc.tensor.matmul(psp[:, i, :], k_dup, qi[:, h, :, :],
                                     start=True, stop=True, perf_mode=DR)
                nc.scalar.activation(p8[:, :, :], psp[:, :, :],
                                     ACTF.Exp, scale=SCALE / (SV * SV),
                                     bias=nbias[:])
                return ('off', p8, kks[0], None)
            kk = kks[0]
            qoff = P * (kk - 4 * jj)
            p16 = p16_pool.tile([P, QT], F16, tag="p16")
            c0 = b * S + kk * P
            k_dup = (kt_sb[:, c0:c0 + P]
                     .rearrange("p (one q) -> p one q", one=1)
                     .broadcast_to([P, 2, P]))
            nc.tensor.matmul(psp[:, 0, qoff:], k_dup, qi[:, h, :, qoff:],
                             start=True, stop=False, perf_mode=DR)
            # causal mask for the diagonal 128x128: one f16 matmul
            # (mask^T stationary x identity) accumulates 0/-65504 into the
            # scores -- keeps the mask off the DVE and out of its queue
            nc.tensor.matmul(psp[:, 0, qoff:qoff + P], mask16[:], ident[:],
                             start=False, stop=True)
            nc.scalar.activation(p16[:, qoff:], psp[:, 0, qoff:],
                                 ACTF.Exp, scale=SCALE / (SV * SV),
                                 bias=nbias[:])
            return ('diag', p16, kk, qoff)

        def finalize_head(h, ps_out, p_sum, ps_db, had_pairs):
            # final 0.5-matmul folds the diagonal (fp16) prob sums into the
            # head's f32 den accumulator (pair units summed there on the PE
            # via stride-0 ones8 DoubleRow); then normalize and split the
            # c_proj input into fp8 hi+lo planes.
            nc.tensor.matmul(ps_db[:], ones_mat[:], p_sum[:],
                             start=not had_pairs, stop=True)
            inv_bc = ibc_pool.tile([P, QT], F16, tag="ibc")
            with nc.allow_low_precision(reason="f16 inv: den spans 2e-3..500, "
                                        "1e-3 rel err ≪ fp8 prob noise"):
                nc.vector.reciprocal(inv_bc[:], ps_db[:])
            at16 = at16_pool.tile([P, QT], F16, tag="a16")
            nc.vector.tensor_mul(at16[:], ps_out[:], inv_bc[:])
            nc.scalar.copy(ati[:, h, 1, :], at16[:])
            nc.gpsimd.tensor_sub(ati[:, h, 0, :], at16[:], ati[:, h, 1, :])

        NU = len(units)
        stream = [(h, u) for h in range(HC) for u in range(NU)]
        total_units = len(stream)
        units_done = 0
        cq_done = sum(cp.hdone for cp in cproj_q)
        cq_start = cq_done
        cq_budget = sum(2 * cp.total for cp in cproj_q) - cq_start
        pending = None
        ps_out = None
        p_sum = None
        u_next = emit_unit(*stream[0])
        # V transposes (fp16) for this q-block, then fp8 hi/lo planes
        for i in range(QT // P):
            tp = ps_acc.tile([P, P], F16, tag="acc")
            nc.tensor.transpose(tp[:], v_st[:, i * P:(i + 1) * P],
                                ident[:])
            mt = j * (QT // P) + i
            nc.vector.tensor_copy(v16_sb[:, mt, :], tp[:])
            nc.scalar.copy(vhi_sb[:, mt, :], v16_sb[:, mt, :])
            nc.gpsimd.tensor_sub(vlo_sb[:, mt, :], v16_sb[:, mt, :],
                                 vhi_sb[:, mt, :])
        if j + 1 < NJ:
            if j == 0:
                xt_next = prefetch_xq(1)
                # w2 is first needed by cproj0 (next block's attention);
                # issuing it after block 1's xq keeps the serial DMA stream
                # feeding the QKV filler first
                nc.scalar.dma_start(out=w2_sb[:], in_=w24[:])
                nc.scalar.dma_start(out=b2_sb[:], in_=b2[:])
            qkv_next = _QKV(j + 1, xt_next)
        ps_db = None
        had_pairs = False
        for idx, (h, u) in enumerate(stream):
            kind, pt, kk0, extra = u_next
            if u == 0 and pending is not None:
                finalize_head(*pending)
                pending = None
            if idx + 1 < total_units:
                u_next = emit_unit(*stream[idx + 1])
            if u == 0:
                ps_out = ps_acc.tile([P, QT], F32, tag="acc")
                p_sum = psum_pool.tile([P, QT], F16, tag="psum")
                ps_db = ps_misc.tile([P, QT], F32, tag="misc")
                had_pairs = False
            # filler BEFORE this unit's PV matmuls (cover the exp latency
            # the PV waits on): previous block's c_proj, then the next
            # block's QKV
            if cproj_q:
                target = cq_start + cq_budget * (units_done + 3) // total_units
                while cq_done < target and cproj_q:
                    if cproj_q[0].half_step():
                        cq_done += 1
                    else:
                        cproj_q.pop(0)
            if qkv_next is not None and units_done > 0:
                target = qkv_next.total_mm * (units_done + 2) // total_units
                while qkv_next.done_mm < target and qkv_next.step(8):
                    pass
            if kind == 'off':
                mt0 = b * (S // P) + kk0
                nc.tensor.matmul(ps_out[:], vhi_sb[:, mt0:mt0 + 2, :],
                                 pt[:, :, :], start=(kk0 == 0), stop=False,
                                 perf_mode=DR)
                nc.tensor.matmul(ps_out[:], vlo_sb[:, mt0:mt0 + 2, :],
                                 pt[:, :, :], start=False, stop=False,
                                 perf_mode=DR)
                o_dup = (ones8[:]
                         .rearrange("p (one q) -> p one q", one=1)
                         .broadcast_to([P, 2, P]))
                nc.tensor.matmul(ps_db[:], o_dup, pt[:, :, :],
                                 start=not had_pairs, stop=False,
                                 perf_mode=DR)
                had_pairs = True
            else:
                kk, qoff = kk0, extra
                nc.tensor.matmul(ps_out[:, qoff:],
                                 v16_sb[:, b * (S // P) + kk, :],
                                 pt[:, qoff:], start=(kk == 0),
                                 stop=(kk == nk - 1))
                if kk == 4 * jj:
                    nc.vector.tensor_copy(p_sum[:], pt[:])
                else:
                    nc.vector.tensor_add(p_sum[:, qoff:], p_sum[:, qoff:],
                                         pt[:, qoff:])
            units_done += 1
            if u == NU - 1:
                pending = (h, ps_out, p_sum, ps_db, had_pairs)
        finalize_head(*pending)
        keep = SPILL if (j + 1 < NJ and (j + 1) % NJB == NJB - 1) else 0
        for ci, cp in enumerate(cproj_q):
            limit = 2 * cp.total - (keep if ci == len(cproj_q) - 1 else 0)
            while cp.hdone < limit and cp.half_step():
                pass
        cproj_q = [cp for cp in cproj_q if cp.hdone < 2 * cp.total]
        cproj_q.append(_CProj(nc, tb, ati, w2_sb, b2_sb, yt3,
                              ps_acc, y_pool, final=(j == NJ - 1)))
    for cp in cproj_q:
        while cp.step():
            pass


_PROGRAM = None


def _get_program():
    global _PROGRAM
    if _PROGRAM is None:
        _PROGRAM = build_program()
    return _PROGRAM


def _split8(a):
    hi = a.astype(E4)
    lo = (a - hi.astype(np.float32)).astype(E4)
    return hi, lo


def make_in_maps(hidden_states, w_qkv, b_qkv, w_proj, b_proj):
    x = np.asarray(hidden_states, dtype=np.float32).reshape(T, D)
    xs = np.ascontiguousarray(x.T) * SX          # [D, T]
    xhi, xlo = _split8(xs)
    xhi_r = xhi.reshape(NKD, P, T)
    xlo_r = xlo.reshape(NKD, P, T)
    xq = np.ascontiguousarray(
        np.concatenate([xlo_r, xhi_r], axis=0).reshape(2 * NKD * P, T))
    pi = np.arange(P)[:, None]
    kk = np.arange(P)[None, :]
    mask = np.where(kk <= pi, 0.0, -65504.0).astype(np.float16)
    mask = np.ascontiguousarray(mask)
    w_qkv = np.asarray(w_qkv, dtype=np.float32)
    b_qkv = np.asarray(b_qkv, dtype=np.float32)
    w_proj = np.asarray(w_proj, dtype=np.float32)
    b_proj = np.asarray(b_proj, dtype=np.float32)
    b2 = np.ascontiguousarray(
        (b_proj / NCORES).reshape(D // P, P).T).astype(np.float32)
    in_maps = []
    for c in range(NCORES):
        qcols = slice(c * DQC, (c + 1) * DQC)
        wsel = np.concatenate([w_qkv[:, qcols], w_qkv[:, D:]], axis=1) * SW
        whi, wlo = _split8(wsel)                  # [D, E1]
        # -> [eb, p, plane, kd, q]; plane0=hi
        w1 = np.stack([whi.reshape(NKD, P, NEB, P),
                       wlo.reshape(NKD, P, NEB, P)], axis=0)
        w1 = w1.transpose(3, 2, 0, 1, 4).reshape(NEB * P, 2 * NKD * P)
        b1 = SV * np.concatenate([b_qkv[qcols], b_qkv[D:]])
        wps = w_proj[c * DQC:(c + 1) * DQC, :] * SW
        w2hi, w2lo = _split8(wps)                 # [DQC, D]
        w2 = np.stack([w2hi.reshape(HC, P, D),
                       w2lo.reshape(HC, P, D)], axis=1).reshape(HC * 2 * P, D)
        in_maps.append({
            "xq": xq,
            "w1": np.ascontiguousarray(w1),
            "b1": np.ascontiguousarray(b1.reshape(NEB, P).T).astype(np.float32),
            "w2": np.ascontiguousarray(w2),
            "b2": b2,
            "mask": mask,
        })
    return in_maps


def kernel(hidden_states, w_qkv, b_qkv, w_proj, b_proj):
    nc = _get_program()
    in_maps = make_in_maps(hidden_states, w_qkv, b_qkv, w_proj, b_proj)
    res = run_bass_kernel_spmd(nc, in_maps, list(range(NCORES)))
    y = np.zeros((D, T), dtype=np.float32)
    for r in res.results:
        y += np.asarray(r["yt"]).astype(np.float32)
    return np.ascontiguousarray(y.T.reshape(B, S, D))


# revision 27
# speedup vs baseline: 1.0467x; 1.0099x over previous
"""GPTBigCode MQA causal attention block on 8 TRN2 NeuronCores — v4.

Tensor-parallel over heads (4 of 32 query heads per core, single KV head
replicated), row-parallel c_proj, bf16 partial outputs summed on host.

v4 = v3 (fp8 DoubleRow GEMMs + fp8 off-diag attention) plus:
- Scores in fp8 DoubleRow at 0.5x: K^T is plain fp8 (x16) duplicated
  across both stationary slots via a stride-0 AP; Q is split hi+lo fp8
  (x16) in the two moving slots, so one DoubleRow computes
  k8^T(q_hi+q_lo) — full Q precision, only K carries plain-fp8 error.
- Engine rebalance (GPSIMD cannot touch PSUM; DVE's 2x path needs all
  operands 2-byte): y evictions alternate DVE/ACT; SBUF-only fp8 ops
  (prob pair combine, q/v/at lo-splits) go to the idle GPSIMD; the
  softmax-denominator accumulation chain is fp16-only on DVE (2x mode)
  fed by per-pair GPSIMD combines that run in parallel.
"""

import numpy as np
from contextlib import ExitStack

import ml_dtypes
import concourse.bass as bass
import concourse.tile as tile
from concourse import bass_isa, mybir
from concourse.bass_utils import run_bass_kernel_spmd
from concourse.masks import make_identity

B, S, D = 2, 2048, 4096
H, DH = 32, 128
NCORES = 8
HC = H // NCORES          # 4 heads per core
DQC = HC * DH             # 512 q-dims per core
T = B * S                 # 4096 tokens
P = 128
NKD = D // P              # 32 contraction tiles in model dim
E1 = DQC + 2 * DH         # 768 per-core QKV output dims
NEB = E1 // P             # 6 e-blocks: 4 Q heads, K, V
QT = 512                  # tokens per (b,j) group
NJ = T // QT              # 8 groups
NJB = S // QT             # 4 groups per batch
SCALE = DH ** -0.5
NKC = NKD // 2            # kd tiles per xq chunk

SX = 32.0                 # x fp8 scale
SW = 2048.0               # weight fp8 scale
SV = 16.0                 # q/k/v scale (fp16 master + fp8)
DEQ = 1.0 / (SX * SW)     # 2^-16
EXPB = -2.0               # exp bias: p~ = e^(s*SCALE - 2)
NQKV = 48                 # DoubleRow instrs per QKV e-block

F32 = mybir.dt.float32
R32 = mybir.dt.float32r
BF16 = mybir.dt.bfloat16
F16 = mybir.dt.float16
FP8 = mybir.dt.float8e4
ACTF = mybir.ActivationFunctionType
DR = mybir.MatmulPerfMode.DoubleRow
ALU = mybir.AluOpType
NEG = -1.0e30
BF = ml_dtypes.bfloat16
E4 = ml_dtypes.float8_e4m3


def build_program():
    nc = bass.Bass()
    # plane-major fp8 x: row index = (plane*NKD + kd)*128 + p, plane0=lo/1=hi
    xq = nc.declare_dram_parameter("xq", [2 * NKD * P, T], FP8, isOutput=False)
    # w1: e-block-major, per e-block columns = (plane, kd, q), plane0=hi/1=lo
    w1 = nc.declare_dram_parameter("w1", [NEB * P, 2 * NKD * P], FP8,
                                   isOutput=False)
    b1 = nc.declare_dram_parameter("b1", [P, NEB], F32, isOutput=False)
    # w2: row index = (kh*2 + plane)*128 + p, plane0=hi/1=lo
    w2 = nc.declare_dram_parameter("w2", [HC * 2 * P, D], FP8, isOutput=False)
    b2 = nc.declare_dram_parameter("b2", [P, D // P], F32, isOutput=False)
    maskp = nc.declare_dram_parameter("mask", [P, P], F16, isOutput=False)
    yt = nc.declare_dram_parameter("yt", [D, T], BF16, isOutput=True)

    with tile.TileContext(nc) as tc:
        with ExitStack() as ctx:
            _body(ctx, tc, nc, xq, w1, b1, w2, b2, maskp, yt)
    _legalize_waits(nc)
    return nc


def _legalize_waits(nc, nop_cap=1):
    """walrus's per-instruction sync-wait budget is tiny for matmuls (LDW+MM
    lowering) and DMA pseudo-instructions. Drop redundant same-engine
    self-waits (engines execute in order), then spill excess waits onto
    same-engine NoOps inserted right before the instruction."""
    nocap = (mybir.InstNoOp,)
    f = nc.m.functions[0]
    for bb in f.blocks:
        insts = bb.instructions
        for i in insts:
            si = i.sync_info
            if si is None or not si.on_wait:
                continue
            ename = str(i.engine).split(".")[-1]
            if ename == "SP":
                ename = "Sync"
            kept = [w for w in si.on_wait
                    if w.sync_type != "semaphore"
                    or w.wait_reg is not None
                    or not w.ant_name.split("_")[0] == ename]
            if len(kept) != len(si.on_wait):
                si.on_wait = kept
        idx = 0
        while idx < len(insts):
            i = insts[idx]
            si = i.sync_info
            cap = None if isinstance(i, nocap) else 1
            if cap is not None and si is not None and len(si.on_wait) > cap:
                excess = list(si.on_wait[:-cap])
                si.on_wait = list(si.on_wait[-cap:])
                while excess:
                    chunk, excess = excess[:nop_cap], excess[nop_cap:]
                    nop = mybir.InstNoOp(
                        name=nc.get_next_instruction_name(), ins=[], outs=[])
                    nop.engine = i.engine
                    nop.sync_info = mybir.SyncInfo(on_wait=chunk, on_update=[])
                    nc.register_instruction(nop)
                    insts.insert(idx, nop)
                    idx += 1
            idx += 1


class _CProj:
    """Stepwise emitter for one q-block's c_proj, interleaved into the NEXT
    q-block's (exp-paced) attention. Per me-tile: 6 DoubleRow matmuls
    (2 hi@hi kh-pairs + 4 cross per-kh); eviction with fused 2^-16 dequant
    and bias alternates DVE / ACT to split the f32-psum read load."""

    LAG = 1

    def __init__(self, nc, tb, ati, w2_sb, b2_sb, yt3, ps_acc, y_pool,
                 final=False):
        self.nc = nc
        self.tb = tb
        self.final = final
        if final:
            self.LAG = 2
        self.ati = ati
        self.w2_sb = w2_sb
        self.b2_sb = b2_sb
        self.yt3 = yt3
        self.ps_acc = ps_acc
        self.y_pool = y_pool
        self.ps_ys = {}
        self.y_t = None
        self.done = 0
        self.hdone = 0
        self.total = D // P + self.LAG

    def step(self):
        return self.half_step() and (self.half_step() or True)

    def half_step(self):
        if self.hdone >= 2 * self.total:
            return False
        me, phase = self.hdone // 2, self.hdone % 2
        self.hdone += 1
        self.done = self.hdone // 2
        nc = self.nc
        NME = D // P
        MG = NME // 8
        if phase == 0 and me >= self.LAG:
            md = me - self.LAG
            ps_y = self.ps_ys.pop(md)
            nc.tensor.matmul(ps_y[:],
                             self.w2_sb[:, HC - 1, :, md * P:(md + 1) * P],
                             self.ati[:, HC - 1, :, :],
                             start=False, stop=True, perf_mode=DR)
            mg, mi = md // MG, md % MG
            if mi == 0:
                y_t = self.y_pool.tile([P, MG, QT], BF16, tag="y")
                self.y_t = y_t
            if True:
                nc.vector.tensor_scalar(self.y_t[:, mi, :], ps_y[:],
                                        DEQ, self.b2_sb[:, md:md + 1],
                                        ALU.mult, ALU.add)
            else:
                nc.scalar.activation(self.y_t[:, mi, :], ps_y[:],
                                     ACTF.Identity, scale=DEQ,
                                     bias=self.b2_sb[:, md:md + 1])
            if self.final and mg == NME // MG - 1:
                nc.sync.dma_start(
                    out=self.yt3[:, mg * MG + mi:mg * MG + mi + 1,
                                 self.tb:self.tb + QT],
                    in_=self.y_t[:, mi:mi + 1, :])
            elif mi == MG - 1:
                nc.sync.dma_start(
                    out=self.yt3[:, mg * MG:(mg + 1) * MG,
                                 self.tb:self.tb + QT],
                    in_=self.y_t[:])
        if phase == 1 and me < NME:
            ps_y = self.ps_acc.tile([P, QT], F32, tag="acc")
            self.ps_ys[me] = ps_y
            cols = slice(me * P, (me + 1) * P)
            nc.tensor.matmul(ps_y[:], self.w2_sb[:, 0:2, 0, cols],
                             self.ati[:, 0:2, 1, :],
                             start=True, stop=False, perf_mode=DR)
            nc.tensor.matmul(ps_y[:], self.w2_sb[:, 2:4, 0, cols],
                             self.ati[:, 2:4, 1, :],
                             start=False, stop=False, perf_mode=DR)
            for kh in range(HC - 1):
                nc.tensor.matmul(ps_y[:], self.w2_sb[:, kh, :, cols],
                                 self.ati[:, kh, :, :],
                                 start=False, stop=False, perf_mode=DR)
        return True


def _body(ctx, tc, nc, xq, w1, b1, w2, b2, maskp, yt):
    xq4 = xq.rearrange("(two kd p) t -> p two kd t", p=P, two=2)
    w13 = w1.rearrange("(eb p) x -> p eb x", p=P)
    w24 = w2.rearrange("(kh two p) d -> p kh two d", p=P, two=2)
    yt3 = yt.rearrange("(me p) t -> p me t", p=P)

    persist = ctx.enter_context(tc.tile_pool(name="persist", bufs=1))
    w1_sb = persist.tile([P, NEB, 2, NKD, P], FP8)   # [d_in, eb, hi/lo, kd, q]
    w2_sb = persist.tile([P, HC, 2, D], FP8)         # [dqc, kh, hi/lo, d_out]
    kt_sb = persist.tile([P, T], FP8)                # 16*K^T [dh, t] plain fp8
    v16_sb = persist.tile([P, T // P, DH], F16)      # 16*V [t_part, mt, dh]
    vio_sb = persist.tile([P, T // P, 2, DH], FP8)   # V fp8 (hi, lo)
    b1_sb = persist.tile([P, NEB], F32)
    b2_sb = persist.tile([P, D // P], F32)
    mask16 = persist.tile([P, P], F16)   # causal mask^T (0/-65504), PE-applied
    nbias = persist.tile([P, 1], F32)                # exp bias -2
    ones_mat = persist.tile([P, P], F16)             # 0.5: folds scales
    ones8 = persist.tile([P, P], FP8)                # 0.5 for fp8 den DR
    ones8q = persist.tile([P, P], FP8)               # 0.25: dup-slot den DR
    ident = persist.tile([P, P], F16)
    nc.vector.memset(ones_mat[:], 0.5)
    nc.vector.memset(ones8[:], 0.5)
    nc.vector.memset(ones8q[:], 0.25)
    nc.vector.memset(nbias[:], EXPB)

    # w1 (per e-block) and the first q-block's xq are queued in the order
    # the first QKV e-block consumes them.
    xt_pool = ctx.enter_context(tc.tile_pool(name="xt", bufs=3))
    xt_first = []
    for _half in range(2):
        xt_c = xt_pool.tile([P, 2, NKC, QT], FP8, tag="xt")
        xt_first.append(xt_c)
    W1C = 2 * NKD * P
    # (kind, eb/chunk, plane): eb0's hi weights + both chunks' hi planes
    # first, so the 16 hi@hi matmuls of eb0 start after ~1.5MB of DMA.
    for kind, a, pl in [
            ('w1', 0, 0), ('xt', 0, 1), ('xt', 1, 1), ('w1', 0, 1),
            ('xt', 0, 0), ('xt', 1, 0), ('w1', 1, 0), ('w1', 1, 1),
            ('w1', 2, None), ('w1', 3, None), ('w1', 4, None),
            ('w1', 5, None)]:
        if kind == 'w1':
            if pl is None:
                nc.sync.dma_start(
                    out=w1_sb[:, a].rearrange("p two kd q -> p (two kd q)"),
                    in_=w13[:, a, :])
            else:
                nc.sync.dma_start(
                    out=w1_sb[:, a, pl], in_=w13[:, a, pl * (W1C // 2):
                                                 (pl + 1) * (W1C // 2)]
                    .rearrange("p (kd q) -> p kd q", q=P))
        else:
            nc.scalar.dma_start(
                out=xt_first[a][:, pl, :, :],
                in_=xq4[:, pl, a * NKC:(a + 1) * NKC, 0:QT])
    nc.scalar.dma_start(out=b1_sb[:], in_=b1[:])
    nc.scalar.dma_start(out=mask16[:], in_=maskp[:])
    make_identity(nc, ident[:])

    # PSUM: 3 (acc) + 2*2 (score pairs) + 1 (misc) = 8 banks
    ps_acc = ctx.enter_context(tc.tile_pool(name="ps_acc", bufs=3, space="PSUM"))
    ps_pair = ctx.enter_context(tc.tile_pool(name="ps_pair", bufs=2, space="PSUM"))
    ps_misc = ctx.enter_context(tc.tile_pool(name="ps_misc", bufs=1, space="PSUM"))

    qt_pool = ctx.enter_context(tc.tile_pool(name="qt", bufs=2))
    q16_pool = ctx.enter_context(tc.tile_pool(name="q16", bufs=2))
    vs_pool = ctx.enter_context(tc.tile_pool(name="vs", bufs=2))
    p_pool = ctx.enter_context(tc.tile_pool(name="pp", bufs=3))
    p16_pool = ctx.enter_context(tc.tile_pool(name="p16", bufs=3))
    psum_pool = ctx.enter_context(tc.tile_pool(name="psm", bufs=2))
    ibc_pool = ctx.enter_context(tc.tile_pool(name="ibc", bufs=2))
    at16_pool = ctx.enter_context(tc.tile_pool(name="a16", bufs=2))
    ati_pool = ctx.enter_context(tc.tile_pool(name="ati", bufs=3))
    y_pool = ctx.enter_context(tc.tile_pool(name="yp", bufs=2))

    class _QKV:
        """Stepwise emitter for one q-block's QKV: per e-block, 16 hi@hi
        DoubleRow (adjacent kd pairs, hi planes) + 32 cross DoubleRow
        (per-kd (w_hi,x_lo)+(w_lo,x_hi)) into one PSUM group."""

        def __init__(self, j, xt_cs):
            self.tb = j * QT
            self.xt_cs = xt_cs
            self.qi = qt_pool.tile([P, HC, 2, QT], FP8, tag="qt")
            self.v_st = None
            self.eb = 0
            self.mi = 0
            self.ps = None
            self.total_mm = NEB * NQKV
            self.done_mm = 0

        def step(self, n_mm=8):
            if self.eb >= NEB:
                return False
            for _ in range(n_mm):
                if self.ps is None:
                    self.ps = ps_acc.tile([P, QT], F32, tag="acc")
                eb, mi = self.eb, self.mi
                if mi < NKD // 2:          # hi@hi: kd pair (2mi, 2mi+1)
                    kd0 = 2 * mi
                    c, r = kd0 // NKC, kd0 % NKC
                    nc.tensor.matmul(
                        self.ps[:], w1_sb[:, eb, 0, kd0:kd0 + 2, :],
                        self.xt_cs[c][:, 1, r:r + 2, :],
                        start=(mi == 0), stop=False, perf_mode=DR)
                else:                      # cross: kd = mi - 16
                    kd = mi - NKD // 2
                    c, r = kd // NKC, kd % NKC
                    nc.tensor.matmul(
                        self.ps[:], w1_sb[:, eb, :, kd, :],
                        self.xt_cs[c][:, :, r, :],
                        start=False, stop=(mi == NQKV - 1), perf_mode=DR)
                self.done_mm += 1
                self.mi += 1
                if self.mi == NQKV:
                    self._evict()
                    self.mi = 0
                    self.eb += 1
                    self.ps = None
                    if self.eb >= NEB:
                        return False
            return True

        def _evict(self):
            eb, ps = self.eb, self.ps
            # b1 is pre-scaled x16 on host for all columns
            if eb < HC:      # Q head: 16*q -> f16 master, then fp8 hi+lo
                q16 = q16_pool.tile([P, QT], F16, tag="q16")
                nc.scalar.activation(q16[:], ps[:],
                                     ACTF.Identity, scale=DEQ * SV,
                                     bias=b1_sb[:, eb:eb + 1])
                nc.scalar.copy(self.qi[:, eb, 1, :], q16[:])
                nc.vector.tensor_sub(self.qi[:, eb, 0, :], q16[:],
                                     self.qi[:, eb, 1, :])
            elif eb == HC:   # K^T: plain fp8 x16
                nc.scalar.activation(kt_sb[:, self.tb:self.tb + QT], ps[:],
                                     ACTF.Identity, scale=DEQ * SV,
                                     bias=b1_sb[:, eb:eb + 1])
            else:            # V: 16*(v+b) -> f16 on DVE
                v_s = vs_pool.tile([P, QT], F16, tag="vs")
                nc.vector.tensor_scalar(v_s[:], ps[:], DEQ * SV,
                                        b1_sb[:, eb:eb + 1],
                                        ALU.mult, ALU.add)
                self.v_st = v_s

    def prefetch_xq(jp):
        cs = []
        for half in range(2):
            xt_c = xt_pool.tile([P, 2, NKC, QT], FP8, tag="xt")
            for pl in (1, 0):
                nc.sync.dma_start(
                    out=xt_c[:, pl, :, :],
                    in_=xq4[:, pl, half * NKC:(half + 1) * NKC,
                             jp * QT:jp * QT + QT])
            cs.append(xt_c)
        return cs

    SPILL = 16     # half-steps of c_proj spilled into the next (jj=3) window
    cproj_q = []
    qkv_cur = None
    qkv_next = None
    xt_next = None
    for j in range(NJ):
        b, jj = j // NJB, j % NJB
        tb = j * QT

        # ---- QKV for tokens [tb, tb+QT) -----------------------------------
        qkv_cur = qkv_next if qkv_next is not None else _QKV(j, xt_first)
        qkv_next = None
        if j + 1 < NJ and j > 0:
            xt_next = prefetch_xq(j + 1)
        while qkv_cur.step():
            pass
        qi = qkv_cur.qi
        v_st = qkv_cur.v_st

        # ---- attention for this q-block (4 heads) -------------------------
        # Scores: one DoubleRow per k-tile (K^T stride-0-duplicated in the
        # stationary slots, q hi+lo in the moving slots). Off-diag pairs:
        # one fp8 exp covers both k-tiles, then 2 DoubleRow PV (vhi, vlo
        # slot-paired across the pair). Diagonal: fp16 probs and fp16 V.
        # Denominator: per-pair GPSIMD combine (fp8+fp8->f16) + fp16-only
        # DVE accumulate chain (2x mode); one 0.5-matmul per head reduces
        # and broadcasts it with the x16 V / x32 at scales folded in.
        ati = ati_pool.tile([P, HC, 2, QT], FP8, tag="ati")  # plane0=lo/1=hi
        nk = 4 * jj + 4
        units = [(kk, kk + 1) for kk in range(0, 4 * jj, 2)] \
            + [(kk,) for kk in range(4 * jj, nk)]

        def emit_unit(h, u):
            kks = units[u]
            psp = ps_pair.tile([P, 2, QT], F32, tag="pair")
            if len(kks) == 2:
                p8 = p_pool.tile([P, 2, QT], FP8, tag="p")
                for i, kk in enumerate(kks):
                    c0 = b * S + kk * P
                    k_dup = (kt_sb[:, c0:c0 + P]
                             .rearrange("p (one q) -> p one q", one=1)
                             .broadcast_to([P, 2, P]))
                    nc.tensor.matmul(psp[:, i, :], k_dup, qi[:, h, :, :],
                                     start=True, stop=True, perf_mode=DR)
                nc.scalar.activation(p8[:, :, :], psp[:, :, :],
                                     ACTF.Exp, scale=SCALE / (SV * SV),
                                     bias=nbias[:])
                return ('off', p8, kks[0], None)
            kk = kks[0]
            qoff = P * (kk - 4 * jj)
            c0 = b * S + kk * P
            k_dup = (kt_sb[:, c0:c0 + P]
                     .rearrange("p (one q) -> p one q", one=1)
                     .broadcast_to([P, 2, P]))
            nc.tensor.matmul(psp[:, 0, qoff:], k_dup, qi[:, h, :, qoff:],
                             start=True, stop=False, perf_mode=DR)
            # causal mask for the diagonal 128x128: one f16 matmul
            # (mask^T stationary x identity) accumulates 0/-65504 into the
            # scores -- keeps the mask off the DVE and out of its queue
            nc.tensor.matmul(psp[:, 0, qoff:qoff + P], mask16[:], ident[:],
                             start=False, stop=True)
            if jj > 0:
                # rows here have >=512 prior keys: fp8 probs are safe, and
                # PV/den can use DoubleRow like the off-diagonal tiles
                p8d = p_pool.tile([P, QT], FP8, tag="p8d")
                nc.scalar.activation(p8d[:, qoff:], psp[:, 0, qoff:],
                                     ACTF.Exp, scale=SCALE / (SV * SV),
                                     bias=nbias[:])
                return ('diag8', p8d, kk, qoff)
            p16 = p16_pool.tile([P, QT], F16, tag="p16")
            nc.scalar.activation(p16[:, qoff:], psp[:, 0, qoff:],
                                 ACTF.Exp, scale=SCALE / (SV * SV),
                                 bias=nbias[:])
            return ('diag', p16, kk, qoff)

        def finalize_head(h, ps_out, p_sum, ps_db, had_pairs):
            # jj==0: final 0.5-matmul folds the fp16 diagonal prob sums into
            # the head's f32 den accumulator; jj>=1 accumulated everything
            # on the PE already (pair ones8-DR + diag dup-slot ones8q-DR).
            if p_sum is not None:
                nc.tensor.matmul(ps_db[:], ones_mat[:], p_sum[:],
                                 start=not had_pairs, stop=True)
            inv_bc = ibc_pool.tile([P, QT], F16, tag="ibc")
            with nc.allow_low_precision(reason="f16 inv: den spans 2e-3..500, "
                                        "1e-3 rel err ≪ fp8 prob noise"):
                nc.vector.reciprocal(inv_bc[:], ps_db[:])
            at16 = at16_pool.tile([P, QT], F16, tag="a16")
            nc.vector.tensor_mul(at16[:], ps_out[:], inv_bc[:])
            nc.scalar.copy(ati[:, h, 1, :], at16[:])
            nc.gpsimd.tensor_sub(ati[:, h, 0, :], at16[:], ati[:, h, 1, :])

        NU = len(units)
        stream = [(h, u) for h in range(HC) for u in range(NU)]
        total_units = len(stream)
        units_done = 0
        cq_done = sum(cp.hdone for cp in cproj_q)
        cq_start = cq_done
        cq_budget = sum(2 * cp.total for cp in cproj_q) - cq_start
        pending = None
        ps_out = None
        p_sum = None
        u_next = emit_unit(*stream[0])
        # V transposes (fp16) for this q-block, then fp8 hi/lo planes
        for i in range(QT // P):
            tp = ps_acc.tile([P, P], F16, tag="acc")
            nc.tensor.transpose(tp[:], v_st[:, i * P:(i + 1) * P],
                                ident[:])
            mt = j * (QT // P) + i
            nc.vector.tensor_copy(v16_sb[:, mt, :], tp[:])
            nc.scalar.copy(vio_sb[:, mt, 0, :], v16_sb[:, mt, :])
            nc.gpsimd.tensor_sub(vio_sb[:, mt, 1, :], v16_sb[:, mt, :],
                                 vio_sb[:, mt, 0, :])
        if j + 1 < NJ:
            if j == 0:
                xt_next = prefetch_xq(1)
                # w2 is first needed by cproj0 (next block's attention);
                # issuing it after block 1's xq keeps the serial DMA stream
                # feeding the QKV filler first
                nc.scalar.dma_start(out=w2_sb[:], in_=w24[:])
                nc.scalar.dma_start(out=b2_sb[:], in_=b2[:])
            qkv_next = _QKV(j + 1, xt_next)
        ps_db = None
        had_pairs = False
        for idx, (h, u) in enumerate(stream):
            kind, pt, kk0, extra = u_next
            if u == 0 and pending is not None:
                finalize_head(*pending)
                pending = None
            if idx + 1 < total_units:
                u_next = emit_unit(*stream[idx + 1])
            if u == 0:
                ps_out = ps_acc.tile([P, QT], F32, tag="acc")
                if jj == 0:
                    p_sum = psum_pool.tile([P, QT], F16, tag="psum")
                else:
                    p_sum = None
                ps_db = ps_misc.tile([P, QT], F32, tag="misc")
                had_pairs = False
            # filler BEFORE this unit's PV matmuls (cover the exp latency
            # the PV waits on): previous block's c_proj, then the next
            # block's QKV
            if cproj_q:
                target = cq_start + cq_budget * (units_done + 3) // total_units
                while cq_done < target and cproj_q:
                    if cproj_q[0].half_step():
                        cq_done += 1
                    else:
                        cproj_q.pop(0)
            if qkv_next is not None and units_done > 0:
                target = qkv_next.total_mm * (units_done + 2) // total_units
                while qkv_next.done_mm < target and qkv_next.step(8):
                    pass
            if kind == 'off':
                mt0 = b * (S // P) + kk0
                nc.tensor.matmul(ps_out[:], vio_sb[:, mt0:mt0 + 2, 0, :],
                                 pt[:, :, :], start=(kk0 == 0), stop=False,
                                 perf_mode=DR)
                nc.tensor.matmul(ps_out[:], vio_sb[:, mt0:mt0 + 2, 1, :],
                                 pt[:, :, :], start=False, stop=False,
                                 perf_mode=DR)
                o_dup = (ones8[:]
                         .rearrange("p (one q) -> p one q", one=1)
                         .broadcast_to([P, 2, P]))
                nc.tensor.matmul(ps_db[:], o_dup, pt[:, :, :],
                                 start=not had_pairs, stop=False,
                                 perf_mode=DR)
                had_pairs = True
            elif kind == 'diag8':
                kk, qoff = kk0, extra
                mt = b * (S // P) + kk
                nw = QT - qoff
                p_dup = (pt[:, qoff:]
                         .rearrange("p (one q) -> p one q", one=1)
                         .broadcast_to([P, 2, nw]))
                nc.tensor.matmul(ps_out[:, qoff:], vio_sb[:, mt, :, :],
                                 p_dup, start=False, stop=(kk == nk - 1),
                                 perf_mode=DR)
                oq_dup = (ones8q[:]
                          .rearrange("p (one q) -> p one q", one=1)
                          .broadcast_to([P, 2, P]))
                nc.tensor.matmul(ps_db[:, qoff:], oq_dup, p_dup,
                                 start=False, stop=(kk == nk - 1),
                                 perf_mode=DR)
            else:
                kk, qoff = kk0, extra
                nc.tensor.matmul(ps_out[:, qoff:],
                                 v16_sb[:, b * (S // P) + kk, :],
                                 pt[:, qoff:], start=(kk == 0),
                                 stop=(kk == nk - 1))
                if kk == 4 * jj:
                    nc.vector.tensor_copy(p_sum[:], pt[:])
                else:
                    nc.vector.tensor_add(p_sum[:, qoff:], p_sum[:, qoff:],
                                         pt[:, qoff:])
            units_done += 1
            if u == NU - 1:
                pending = (h, ps_out, p_sum, ps_db, had_pairs)
        finalize_head(*pending)
        keep = SPILL if (j + 1 < NJ and (j + 1) % NJB == NJB - 1) else 0
        for ci, cp in enumerate(cproj_q):
            limit = 2 * cp.total - (keep if ci == len(cproj_q) - 1 else 0)
            while cp.hdone < limit and cp.half_step():
                pass
        cproj_q = [cp for cp in cproj_q if cp.hdone < 2 * cp.total]
        cproj_q.append(_CProj(nc, tb, ati, w2_sb, b2_sb, yt3,
                              ps_acc, y_pool, final=(j == NJ - 1)))
    for cp in cproj_q:
        while cp.step():
            pass


_PROGRAM = None


def _get_program():
    global _PROGRAM
    if _PROGRAM is None:
        _PROGRAM = build_program()
    return _PROGRAM


def _split8(a):
    hi = a.astype(E4)
    lo = (a - hi.astype(np.float32)).astype(E4)
    return hi, lo


def make_in_maps(hidden_states, w_qkv, b_qkv, w_proj, b_proj):
    x = np.asarray(hidden_states, dtype=np.float32).reshape(T, D)
    xs = np.ascontiguousarray(x.T) * SX          # [D, T]
    xhi, xlo = _split8(xs)
    xhi_r = xhi.reshape(NKD, P, T)
    xlo_r = xlo.reshape(NKD, P, T)
    xq = np.ascontiguousarray(
        np.concatenate([xlo_r, xhi_r], axis=0).reshape(2 * NKD * P, T))
    pi = np.arange(P)[:, None]
    kk = np.arange(P)[None, :]
    mask = np.where(kk <= pi, 0.0, -65504.0).astype(np.float16)
    mask = np.ascontiguousarray(mask)
    w_qkv = np.asarray(w_qkv, dtype=np.float32)
    b_qkv = np.asarray(b_qkv, dtype=np.float32)
    w_proj = np.asarray(w_proj, dtype=np.float32)
    b_proj = np.asarray(b_proj, dtype=np.float32)
    b2 = np.ascontiguousarray(
        (b_proj / NCORES).reshape(D // P, P).T).astype(np.float32)
    in_maps = []
    for c in range(NCORES):
        qcols = slice(c * DQC, (c + 1) * DQC)
        wsel = np.concatenate([w_qkv[:, qcols], w_qkv[:, D:]], axis=1) * SW
        whi, wlo = _split8(wsel)                  # [D, E1]
        # -> [eb, p, plane, kd, q]; plane0=hi
        w1 = np.stack([whi.reshape(NKD, P, NEB, P),
                       wlo.reshape(NKD, P, NEB, P)], axis=0)
        w1 = w1.transpose(3, 2, 0, 1, 4).reshape(NEB * P, 2 * NKD * P)
        b1 = SV * np.concatenate([b_qkv[qcols], b_qkv[D:]])
        wps = w_proj[c * DQC:(c + 1) * DQC, :] * SW
        w2hi, w2lo = _split8(wps)                 # [DQC, D]
        w2 = np.stack([w2hi.reshape(HC, P, D),
                       w2lo.reshape(HC, P, D)], axis=1).reshape(HC * 2 * P, D)
        in_maps.append({
            "xq": xq,
            "w1": np.ascontiguousarray(w1),
            "b1": np.ascontiguousarray(b1.reshape(NEB, P).T).astype(np.float32),
            "w2": np.ascontiguousarray(w2),
            "b2": b2,
            "mask": mask,
        })
    return in_maps


def kernel(hidden_states, w_qkv, b_qkv, w_proj, b_proj):
    nc = _get_program()
    in_maps = make_in_maps(hidden_states, w_qkv, b_qkv, w_proj, b_proj)
    res = run_bass_kernel_spmd(nc, in_maps, list(range(NCORES)))
    y = np.zeros((D, T), dtype=np.float32)
    for r in res.results:
        y += np.asarray(r["yt"]).astype(np.float32)
    return np.ascontiguousarray(y.T.reshape(B, S, D))


# revision 29
# speedup vs baseline: 1.0478x; 1.0011x over previous
"""GPTBigCode MQA causal attention block on 8 TRN2 NeuronCores — v4.

Tensor-parallel over heads (4 of 32 query heads per core, single KV head
replicated), row-parallel c_proj, bf16 partial outputs summed on host.

v4 = v3 (fp8 DoubleRow GEMMs + fp8 off-diag attention) plus:
- Scores in fp8 DoubleRow at 0.5x: K^T is plain fp8 (x16) duplicated
  across both stationary slots via a stride-0 AP; Q is split hi+lo fp8
  (x16) in the two moving slots, so one DoubleRow computes
  k8^T(q_hi+q_lo) — full Q precision, only K carries plain-fp8 error.
- Engine rebalance (GPSIMD cannot touch PSUM; DVE's 2x path needs all
  operands 2-byte): y evictions alternate DVE/ACT; SBUF-only fp8 ops
  (prob pair combine, q/v/at lo-splits) go to the idle GPSIMD; the
  softmax-denominator accumulation chain is fp16-only on DVE (2x mode)
  fed by per-pair GPSIMD combines that run in parallel.
"""

import numpy as np
from contextlib import ExitStack

import ml_dtypes
import concourse.bass as bass
import concourse.tile as tile
from concourse import bass_isa, mybir
from concourse.bass_utils import run_bass_kernel_spmd
from concourse.masks import make_identity

B, S, D = 2, 2048, 4096
H, DH = 32, 128
NCORES = 8
HC = H // NCORES          # 4 heads per core
DQC = HC * DH             # 512 q-dims per core
T = B * S                 # 4096 tokens
P = 128
NKD = D // P              # 32 contraction tiles in model dim
E1 = DQC + 2 * DH         # 768 per-core QKV output dims
NEB = E1 // P             # 6 e-blocks: 4 Q heads, K, V
QT = 512                  # tokens per (b,j) group
NJ = T // QT              # 8 groups
NJB = S // QT             # 4 groups per batch
SCALE = DH ** -0.5
NKC = NKD // 2            # kd tiles per xq chunk

SX = 32.0                 # x fp8 scale
SW = 2048.0               # weight fp8 scale
SV = 16.0                 # q/k/v scale (fp16 master + fp8)
DEQ = 1.0 / (SX * SW)     # 2^-16
EXPB = -2.0               # exp bias: p~ = e^(s*SCALE - 2)
NQKV = 48                 # DoubleRow instrs per QKV e-block

F32 = mybir.dt.float32
R32 = mybir.dt.float32r
BF16 = mybir.dt.bfloat16
F16 = mybir.dt.float16
FP8 = mybir.dt.float8e4
ACTF = mybir.ActivationFunctionType
DR = mybir.MatmulPerfMode.DoubleRow
ALU = mybir.AluOpType
NEG = -1.0e30
BF = ml_dtypes.bfloat16
E4 = ml_dtypes.float8_e4m3


def build_program():
    nc = bass.Bass()
    # plane-major fp8 x: row index = (plane*NKD + kd)*128 + p, plane0=lo/1=hi
    xq = nc.declare_dram_parameter("xq", [2 * NKD * P, T], FP8, isOutput=False)
    # w1: e-block-major, per e-block columns = (plane, kd, q), plane0=hi/1=lo
    w1 = nc.declare_dram_parameter("w1", [NEB * P, 2 * NKD * P], FP8,
                                   isOutput=False)
    b1 = nc.declare_dram_parameter("b1", [P, NEB], F32, isOutput=False)
    # w2: row index = (kh*2 + plane)*128 + p, plane0=hi/1=lo
    w2 = nc.declare_dram_parameter("w2", [HC * 2 * P, D], FP8, isOutput=False)
    b2 = nc.declare_dram_parameter("b2", [P, D // P], F32, isOutput=False)
    maskp = nc.declare_dram_parameter("mask", [P, P], F16, isOutput=False)
    yt = nc.declare_dram_parameter("yt", [D, T], BF16, isOutput=True)

    with tile.TileContext(nc) as tc:
        with ExitStack() as ctx:
            _body(ctx, tc, nc, xq, w1, b1, w2, b2, maskp, yt)
    _legalize_waits(nc)
    return nc


def _legalize_waits(nc, nop_cap=1):
    """walrus's per-instruction sync-wait budget is tiny for matmuls (LDW+MM
    lowering) and DMA pseudo-instructions. Drop redundant same-engine
    self-waits (engines execute in order), then spill excess waits onto
    same-engine NoOps inserted right before the instruction."""
    nocap = (mybir.InstNoOp,)
    f = nc.m.functions[0]
    for bb in f.blocks:
        insts = bb.instructions
        for i in insts:
            si = i.sync_info
            if si is None or not si.on_wait:
                continue
            ename = str(i.engine).split(".")[-1]
            if ename == "SP":
                ename = "Sync"
            kept = [w for w in si.on_wait
                    if w.sync_type != "semaphore"
                    or w.wait_reg is not None
                    or not w.ant_name.split("_")[0] == ename]
            if len(kept) != len(si.on_wait):
                si.on_wait = kept
        idx = 0
        while idx < len(insts):
            i = insts[idx]
            si = i.sync_info
            cap = None if isinstance(i, nocap) else 1
            if cap is not None and si is not None and len(si.on_wait) > cap:
                excess = list(si.on_wait[:-cap])
                si.on_wait = list(si.on_wait[-cap:])
                while excess:
                    chunk, excess = excess[:nop_cap], excess[nop_cap:]
                    nop = mybir.InstNoOp(
                        name=nc.get_next_instruction_name(), ins=[], outs=[])
                    nop.engine = i.engine
                    nop.sync_info = mybir.SyncInfo(on_wait=chunk, on_update=[])
                    nc.register_instruction(nop)
                    insts.insert(idx, nop)
                    idx += 1
            idx += 1


class _CProj:
    """Stepwise emitter for one q-block's c_proj, interleaved into the NEXT
    q-block's (exp-paced) attention. Per me-tile: 6 DoubleRow matmuls
    (2 hi@hi kh-pairs + 4 cross per-kh); eviction with fused 2^-16 dequant
    and bias alternates DVE / ACT to split the f32-psum read load."""

    LAG = 1

    def __init__(self, nc, tb, ati, w2_sb, b2_sb, yt3, ps_acc, y_pool,
                 final=False):
        self.nc = nc
        self.tb = tb
        self.final = final
        if final:
            self.LAG = 2
        self.ati = ati
        self.w2_sb = w2_sb
        self.b2_sb = b2_sb
        self.yt3 = yt3
        self.ps_acc = ps_acc
        self.y_pool = y_pool
        self.ps_ys = {}
        self.y_t = None
        self.done = 0
        self.hdone = 0
        self.total = D // P + self.LAG

    def step(self):
        return self.half_step() and (self.half_step() or True)

    def half_step(self):
        if self.hdone >= 2 * self.total:
            return False
        me, phase = self.hdone // 2, self.hdone % 2
        self.hdone += 1
        self.done = self.hdone // 2
        nc = self.nc
        NME = D // P
        MG = NME // 8
        if phase == 0 and me >= self.LAG:
            md = me - self.LAG
            ps_y = self.ps_ys.pop(md)
            nc.tensor.matmul(ps_y[:],
                             self.w2_sb[:, HC - 1, :, md * P:(md + 1) * P],
                             self.ati[:, HC - 1, :, :],
                             start=False, stop=True, perf_mode=DR)
            mg, mi = md // MG, md % MG
            if mi == 0:
                y_t = self.y_pool.tile([P, MG, QT], BF16, tag="y")
                self.y_t = y_t
            if True:
                nc.vector.tensor_scalar(self.y_t[:, mi, :], ps_y[:],
                                        DEQ, self.b2_sb[:, md:md + 1],
                                        ALU.mult, ALU.add)
            else:
                nc.scalar.activation(self.y_t[:, mi, :], ps_y[:],
                                     ACTF.Identity, scale=DEQ,
                                     bias=self.b2_sb[:, md:md + 1])
            if self.final and mg == NME // MG - 1:
                nc.sync.dma_start(
                    out=self.yt3[:, mg * MG + mi:mg * MG + mi + 1,
                                 self.tb:self.tb + QT],
                    in_=self.y_t[:, mi:mi + 1, :])
            elif mi == MG - 1:
                nc.sync.dma_start(
                    out=self.yt3[:, mg * MG:(mg + 1) * MG,
                                 self.tb:self.tb + QT],
                    in_=self.y_t[:])
        if phase == 1 and me < NME:
            ps_y = self.ps_acc.tile([P, QT], F32, tag="acc")
            self.ps_ys[me] = ps_y
            cols = slice(me * P, (me + 1) * P)
            nc.tensor.matmul(ps_y[:], self.w2_sb[:, 0:2, 0, cols],
                             self.ati[:, 0:2, 1, :],
                             start=True, stop=False, perf_mode=DR)
            nc.tensor.matmul(ps_y[:], self.w2_sb[:, 2:4, 0, cols],
                             self.ati[:, 2:4, 1, :],
                             start=False, stop=False, perf_mode=DR)
            for kh in range(HC - 1):
                nc.tensor.matmul(ps_y[:], self.w2_sb[:, kh, :, cols],
                                 self.ati[:, kh, :, :],
                                 start=False, stop=False, perf_mode=DR)
        return True


def _body(ctx, tc, nc, xq, w1, b1, w2, b2, maskp, yt):
    xq4 = xq.rearrange("(two kd p) t -> p two kd t", p=P, two=2)
    w13 = w1.rearrange("(eb p) x -> p eb x", p=P)
    w24 = w2.rearrange("(kh two p) d -> p kh two d", p=P, two=2)
    yt3 = yt.rearrange("(me p) t -> p me t", p=P)

    persist = ctx.enter_context(tc.tile_pool(name="persist", bufs=1))
    w1_sb = persist.tile([P, NEB, 2, NKD, P], FP8)   # [d_in, eb, hi/lo, kd, q]
    w2_sb = persist.tile([P, HC, 2, D], FP8)         # [dqc, kh, hi/lo, d_out]
    kt_sb = persist.tile([P, T], FP8)                # 16*K^T [dh, t] plain fp8
    v16_sb = persist.tile([P, T // P, DH], F16)      # 16*V [t_part, mt, dh]
    vio_sb = persist.tile([P, T // P, 2, DH], FP8)   # V fp8 (hi, lo)
    b1_sb = persist.tile([P, NEB], F32)
    b2_sb = persist.tile([P, D // P], F32)
    mask16 = persist.tile([P, P], F16)   # causal mask^T (0/-65504), PE-applied
    nbias = persist.tile([P, 1], F32)                # exp bias -2
    ones_mat = persist.tile([P, P], F16)             # 0.5: folds scales
    ones8 = persist.tile([P, P], FP8)                # 0.5 for fp8 den DR
    ones8q = persist.tile([P, P], FP8)               # 0.25: dup-slot den DR
    ident = persist.tile([P, P], F16)
    nc.vector.memset(ones_mat[:], 0.5)
    nc.vector.memset(ones8[:], 0.5)
    nc.vector.memset(ones8q[:], 0.25)
    nc.vector.memset(nbias[:], EXPB)

    # w1 (per e-block) and the first q-block's xq are queued in the order
    # the first QKV e-block consumes them.
    xt_pool = ctx.enter_context(tc.tile_pool(name="xt", bufs=3))
    xt_first = []
    for _half in range(2):
        xt_c = xt_pool.tile([P, 2, NKC, QT], FP8, tag="xt")
        xt_first.append(xt_c)
    W1C = 2 * NKD * P
    # (kind, eb/chunk, plane): eb0's hi weights + both chunks' hi planes
    # first, so the 16 hi@hi matmuls of eb0 start after ~1.5MB of DMA.
    for kind, a, pl in [
            ('w1', 0, 0), ('xt', 0, 1), ('xt', 1, 1), ('w1', 0, 1),
            ('xt', 0, 0), ('xt', 1, 0), ('w1', 1, 0), ('w1', 1, 1),
            ('w1', 2, None), ('w1', 3, None), ('w1', 4, None),
            ('w1', 5, None)]:
        if kind == 'w1':
            if pl is None:
                nc.sync.dma_start(
                    out=w1_sb[:, a].rearrange("p two kd q -> p (two kd q)"),
                    in_=w13[:, a, :])
            else:
                nc.sync.dma_start(
                    out=w1_sb[:, a, pl], in_=w13[:, a, pl * (W1C // 2):
                                                 (pl + 1) * (W1C // 2)]
                    .rearrange("p (kd q) -> p kd q", q=P))
        else:
            nc.scalar.dma_start(
                out=xt_first[a][:, pl, :, :],
                in_=xq4[:, pl, a * NKC:(a + 1) * NKC, 0:QT])
    nc.scalar.dma_start(out=b1_sb[:], in_=b1[:])
    nc.scalar.dma_start(out=mask16[:], in_=maskp[:])
    make_identity(nc, ident[:])

    # PSUM: 3 (acc) + 2*2 (score pairs) + 1 (misc) = 8 banks
    ps_acc = ctx.enter_context(tc.tile_pool(name="ps_acc", bufs=3, space="PSUM"))
    ps_pair = ctx.enter_context(tc.tile_pool(name="ps_pair", bufs=2, space="PSUM"))
    ps_misc = ctx.enter_context(tc.tile_pool(name="ps_misc", bufs=1, space="PSUM"))

    qt_pool = ctx.enter_context(tc.tile_pool(name="qt", bufs=2))
    q16_pool = ctx.enter_context(tc.tile_pool(name="q16", bufs=2))
    vs_pool = ctx.enter_context(tc.tile_pool(name="vs", bufs=2))
    p_pool = ctx.enter_context(tc.tile_pool(name="pp", bufs=3))
    p16_pool = ctx.enter_context(tc.tile_pool(name="p16", bufs=3))
    psum_pool = ctx.enter_context(tc.tile_pool(name="psm", bufs=2))
    ibc_pool = ctx.enter_context(tc.tile_pool(name="ibc", bufs=2))
    at16_pool = ctx.enter_context(tc.tile_pool(name="a16", bufs=2))
    ati_pool = ctx.enter_context(tc.tile_pool(name="ati", bufs=3))
    y_pool = ctx.enter_context(tc.tile_pool(name="yp", bufs=2))

    class _QKV:
        """Stepwise emitter for one q-block's QKV: per e-block, 16 hi@hi
        DoubleRow (adjacent kd pairs, hi planes) + 32 cross DoubleRow
        (per-kd (w_hi,x_lo)+(w_lo,x_hi)) into one PSUM group."""

        def __init__(self, j, xt_cs):
            self.tb = j * QT
            self.xt_cs = xt_cs
            self.qi = qt_pool.tile([P, HC, 2, QT], FP8, tag="qt")
            self.v_st = None
            self.eb = 0
            self.mi = 0
            self.ps = None
            self.total_mm = NEB * NQKV
            self.done_mm = 0

        def step(self, n_mm=8):
            if self.eb >= NEB:
                return False
            for _ in range(n_mm):
                if self.ps is None:
                    self.ps = ps_acc.tile([P, QT], F32, tag="acc")
                eb, mi = self.eb, self.mi
                if mi < NKD // 2:          # hi@hi: kd pair (2mi, 2mi+1)
                    kd0 = 2 * mi
                    c, r = kd0 // NKC, kd0 % NKC
                    nc.tensor.matmul(
                        self.ps[:], w1_sb[:, eb, 0, kd0:kd0 + 2, :],
                        self.xt_cs[c][:, 1, r:r + 2, :],
                        start=(mi == 0), stop=False, perf_mode=DR)
                else:                      # cross: kd = mi - 16
                    kd = mi - NKD // 2
                    c, r = kd // NKC, kd % NKC
                    nc.tensor.matmul(
                        self.ps[:], w1_sb[:, eb, :, kd, :],
                        self.xt_cs[c][:, :, r, :],
                        start=False, stop=(mi == NQKV - 1), perf_mode=DR)
                self.done_mm += 1
                self.mi += 1
                if self.mi == NQKV:
                    self._evict()
                    self.mi = 0
                    self.eb += 1
                    self.ps = None
                    if self.eb >= NEB:
                        return False
            return True

        def _evict(self):
            eb, ps = self.eb, self.ps
            # b1 is pre-scaled x16 on host for all columns
            if eb < HC:      # Q head: 16*q -> f16 master, then fp8 hi+lo
                q16 = q16_pool.tile([P, QT], F16, tag="q16")
                nc.scalar.activation(q16[:], ps[:],
                                     ACTF.Identity, scale=DEQ * SV,
                                     bias=b1_sb[:, eb:eb + 1])
                nc.scalar.copy(self.qi[:, eb, 1, :], q16[:])
                nc.vector.tensor_sub(self.qi[:, eb, 0, :], q16[:],
                                     self.qi[:, eb, 1, :])
            elif eb == HC:   # K^T: plain fp8 x16
                nc.scalar.activation(kt_sb[:, self.tb:self.tb + QT], ps[:],
                                     ACTF.Identity, scale=DEQ * SV,
                                     bias=b1_sb[:, eb:eb + 1])
            else:            # V: 16*(v+b) -> f16 on DVE
                v_s = vs_pool.tile([P, QT], F16, tag="vs")
                nc.vector.tensor_scalar(v_s[:], ps[:], DEQ * SV,
                                        b1_sb[:, eb:eb + 1],
                                        ALU.mult, ALU.add)
                self.v_st = v_s

    def prefetch_xq(jp):
        cs = []
        for half in range(2):
            xt_c = xt_pool.tile([P, 2, NKC, QT], FP8, tag="xt")
            for pl in (1, 0):
                nc.sync.dma_start(
                    out=xt_c[:, pl, :, :],
                    in_=xq4[:, pl, half * NKC:(half + 1) * NKC,
                             jp * QT:jp * QT + QT])
            cs.append(xt_c)
        return cs

    SPILL = 16     # half-steps of c_proj spilled into the next (jj=3) window
    cproj_q = []
    qkv_cur = None
    qkv_next = None
    xt_next = None
    for j in range(NJ):
        b, jj = j // NJB, j % NJB
        tb = j * QT

        # ---- QKV for tokens [tb, tb+QT) -----------------------------------
        qkv_cur = qkv_next if qkv_next is not None else _QKV(j, xt_first)
        qkv_next = None
        if j + 1 < NJ and j > 0:
            xt_next = prefetch_xq(j + 1)
        while qkv_cur.step():
            pass
        qi = qkv_cur.qi
        v_st = qkv_cur.v_st

        # ---- attention for this q-block (4 heads) -------------------------
        # Scores: one DoubleRow per k-tile (K^T stride-0-duplicated in the
        # stationary slots, q hi+lo in the moving slots). Off-diag pairs:
        # one fp8 exp covers both k-tiles, then 2 DoubleRow PV (vhi, vlo
        # slot-paired across the pair). Diagonal: fp16 probs and fp16 V.
        # Denominator: per-pair GPSIMD combine (fp8+fp8->f16) + fp16-only
        # DVE accumulate chain (2x mode); one 0.5-matmul per head reduces
        # and broadcasts it with the x16 V / x32 at scales folded in.
        ati = ati_pool.tile([P, HC, 2, QT], FP8, tag="ati")  # plane0=lo/1=hi
        nk = 4 * jj + 4
        units = [(kk, kk + 1) for kk in range(0, 4 * jj, 2)] \
            + [(kk,) for kk in range(4 * jj, nk)]

        def emit_unit(h, u):
            kks = units[u]
            psp = ps_pair.tile([P, 2, QT], F32, tag="pair")
            if len(kks) == 2:
                p8 = p_pool.tile([P, 2, QT], FP8, tag="p")
                for i, kk in enumerate(kks):
                    c0 = b * S + kk * P
                    k_dup = (kt_sb[:, c0:c0 + P]
                             .rearrange("p (one q) -> p one q", one=1)
                             .broadcast_to([P, 2, P]))
                    nc.tensor.matmul(psp[:, i, :], k_dup, qi[:, h, :, :],
                                     start=True, stop=True, perf_mode=DR)
                nc.scalar.activation(p8[:, :, :], psp[:, :, :],
                                     ACTF.Exp, scale=SCALE / (SV * SV),
                                     bias=nbias[:])
                return ('off', p8, kks[0], None)
            kk = kks[0]
            qoff = P * (kk - 4 * jj)
            c0 = b * S + kk * P
            k_dup = (kt_sb[:, c0:c0 + P]
                     .rearrange("p (one q) -> p one q", one=1)
                     .broadcast_to([P, 2, P]))
            nc.tensor.matmul(psp[:, 0, qoff:], k_dup, qi[:, h, :, qoff:],
                             start=True, stop=False, perf_mode=DR)
            # causal mask for the diagonal 128x128: one f16 matmul
            # (mask^T stationary x identity) accumulates 0/-65504 into the
            # scores -- keeps the mask off the DVE and out of its queue
            nc.tensor.matmul(psp[:, 0, qoff:qoff + P], mask16[:], ident[:],
                             start=False, stop=True)
            if jj > 0 or kk > 0:
                # fp8-fragile rows (few prior keys) are only touched by the
                # kk==0 diagonal tile; every other tile's rows have >=128
                # prior keys, so fp8 probs are safe and PV/den can use
                # DoubleRow like the off-diagonal tiles
                p8d = p_pool.tile([P, QT], FP8, tag="p8d")
                nc.scalar.activation(p8d[:, qoff:], psp[:, 0, qoff:],
                                     ACTF.Exp, scale=SCALE / (SV * SV),
                                     bias=nbias[:])
                return ('diag8', p8d, kk, qoff)
            p16 = p16_pool.tile([P, QT], F16, tag="p16")
            nc.scalar.activation(p16[:, qoff:], psp[:, 0, qoff:],
                                 ACTF.Exp, scale=SCALE / (SV * SV),
                                 bias=nbias[:])
            return ('diag', p16, kk, qoff)

        def finalize_head(h, ps_out, p_sum, ps_db, had_pairs):
            # jj==0: final 0.5-matmul folds the fp16 diagonal prob sums into
            # the head's f32 den accumulator; jj>=1 accumulated everything
            # on the PE already (pair ones8-DR + diag dup-slot ones8q-DR).
            if p_sum is not None:
                nc.tensor.matmul(ps_db[:], ones_mat[:], p_sum[:],
                                 start=not had_pairs, stop=True)
            inv_bc = ibc_pool.tile([P, QT], F16, tag="ibc")
            with nc.allow_low_precision(reason="f16 inv: den spans 2e-3..500, "
                                        "1e-3 rel err ≪ fp8 prob noise"):
                nc.vector.reciprocal(inv_bc[:], ps_db[:])
            at16 = at16_pool.tile([P, QT], F16, tag="a16")
            nc.vector.tensor_mul(at16[:], ps_out[:], inv_bc[:])
            nc.scalar.copy(ati[:, h, 1, :], at16[:])
            nc.gpsimd.tensor_sub(ati[:, h, 0, :], at16[:], ati[:, h, 1, :])

        NU = len(units)
        stream = [(h, u) for h in range(HC) for u in range(NU)]
        total_units = len(stream)
        units_done = 0
        cq_done = sum(cp.hdone for cp in cproj_q)
        cq_start = cq_done
        cq_budget = sum(2 * cp.total for cp in cproj_q) - cq_start
        pending = None
        ps_out = None
        p_sum = None
        u_next = emit_unit(*stream[0])
        # V transposes (fp16) for this q-block, then fp8 hi/lo planes
        for i in range(QT // P):
            tp = ps_acc.tile([P, P], F16, tag="acc")
            nc.tensor.transpose(tp[:], v_st[:, i * P:(i + 1) * P],
                                ident[:])
            mt = j * (QT // P) + i
            nc.vector.tensor_copy(v16_sb[:, mt, :], tp[:])
            nc.scalar.copy(vio_sb[:, mt, 0, :], v16_sb[:, mt, :])
            nc.gpsimd.tensor_sub(vio_sb[:, mt, 1, :], v16_sb[:, mt, :],
                                 vio_sb[:, mt, 0, :])
        if j + 1 < NJ:
            if j == 0:
                xt_next = prefetch_xq(1)
                # w2 is first needed by cproj0 (next block's attention);
                # issuing it after block 1's xq keeps the serial DMA stream
                # feeding the QKV filler first
                nc.scalar.dma_start(out=w2_sb[:], in_=w24[:])
                nc.scalar.dma_start(out=b2_sb[:], in_=b2[:])
            qkv_next = _QKV(j + 1, xt_next)
        ps_db = None
        had_pairs = False
        for idx, (h, u) in enumerate(stream):
            kind, pt, kk0, extra = u_next
            if u == 0 and pending is not None:
                finalize_head(*pending)
                pending = None
            if idx + 1 < total_units:
                u_next = emit_unit(*stream[idx + 1])
            if u == 0:
                ps_out = ps_acc.tile([P, QT], F32, tag="acc")
                if jj == 0:
                    p_sum = psum_pool.tile([P, QT], F16, tag="psum")
                else:
                    p_sum = None
                ps_db = ps_misc.tile([P, QT], F32, tag="misc")
                had_pairs = False
            # filler BEFORE this unit's PV matmuls (cover the exp latency
            # the PV waits on): previous block's c_proj, then the next
            # block's QKV
            if cproj_q:
                target = cq_start + cq_budget * (units_done + 3) // total_units
                while cq_done < target and cproj_q:
                    if cproj_q[0].half_step():
                        cq_done += 1
                    else:
                        cproj_q.pop(0)
            if qkv_next is not None and units_done > 0:
                target = qkv_next.total_mm * (units_done + 2) // total_units
                while qkv_next.done_mm < target and qkv_next.step(8):
                    pass
            if kind == 'off':
                mt0 = b * (S // P) + kk0
                nc.tensor.matmul(ps_out[:], vio_sb[:, mt0:mt0 + 2, 0, :],
                                 pt[:, :, :], start=(kk0 == 0), stop=False,
                                 perf_mode=DR)
                nc.tensor.matmul(ps_out[:], vio_sb[:, mt0:mt0 + 2, 1, :],
                                 pt[:, :, :], start=False, stop=False,
                                 perf_mode=DR)
                o_dup = (ones8[:]
                         .rearrange("p (one q) -> p one q", one=1)
                         .broadcast_to([P, 2, P]))
                nc.tensor.matmul(ps_db[:], o_dup, pt[:, :, :],
                                 start=not had_pairs, stop=False,
                                 perf_mode=DR)
                had_pairs = True
            elif kind == 'diag8':
                kk, qoff = kk0, extra
                mt = b * (S // P) + kk
                nw = QT - qoff
                p_dup = (pt[:, qoff:]
                         .rearrange("p (one q) -> p one q", one=1)
                         .broadcast_to([P, 2, nw]))
                nc.tensor.matmul(ps_out[:, qoff:], vio_sb[:, mt, :, :],
                                 p_dup, start=False, stop=(kk == nk - 1),
                                 perf_mode=DR)
                oq_dup = (ones8q[:]
                          .rearrange("p (one q) -> p one q", one=1)
                          .broadcast_to([P, 2, P]))
                if not had_pairs:
                    # jj==0 head: this den-DR opens the ps_db group; zero
                    # the unwritten prefix and cover the full width
                    nc.vector.memset(pt[:, 0:qoff], 0.0)
                    pd_full = (pt[:]
                               .rearrange("p (one q) -> p one q", one=1)
                               .broadcast_to([P, 2, QT]))
                    nc.tensor.matmul(ps_db[:], oq_dup, pd_full,
                                     start=True, stop=(kk == nk - 1),
                                     perf_mode=DR)
                else:
                    nc.tensor.matmul(ps_db[:, qoff:], oq_dup, p_dup,
                                     start=False, stop=(kk == nk - 1),
                                     perf_mode=DR)
                had_pairs = True
            else:
                kk, qoff = kk0, extra
                nc.tensor.matmul(ps_out[:, qoff:],
                                 v16_sb[:, b * (S // P) + kk, :],
                                 pt[:, qoff:], start=(kk == 0),
                                 stop=(kk == nk - 1))
                if kk == 4 * jj:
                    nc.vector.tensor_copy(p_sum[:], pt[:])
                else:
                    nc.vector.tensor_add(p_sum[:, qoff:], p_sum[:, qoff:],
                                         pt[:, qoff:])
            units_done += 1
            if u == NU - 1:
                pending = (h, ps_out, p_sum, ps_db, had_pairs)
        finalize_head(*pending)
        keep = SPILL if (j + 1 < NJ and (j + 1) % NJB == NJB - 1) else 0
        for ci, cp in enumerate(cproj_q):
            limit = 2 * cp.total - (keep if ci == len(cproj_q) - 1 else 0)
            while cp.hdone < limit and cp.half_step():
                pass
        cproj_q = [cp for cp in cproj_q if cp.hdone < 2 * cp.total]
        cproj_q.append(_CProj(nc, tb, ati, w2_sb, b2_sb, yt3,
                              ps_acc, y_pool, final=(j == NJ - 1)))
    for cp in cproj_q:
        while cp.step():
            pass


_PROGRAM = None


def _get_program():
    global _PROGRAM
    if _PROGRAM is None:
        _PROGRAM = build_program()
    return _PROGRAM


def _split8(a):
    hi = a.astype(E4)
    lo = (a - hi.astype(np.float32)).astype(E4)
    return hi, lo


def make_in_maps(hidden_states, w_qkv, b_qkv, w_proj, b_proj):
    x = np.asarray(hidden_states, dtype=np.float32).reshape(T, D)
    xs = np.ascontiguousarray(x.T) * SX          # [D, T]
    xhi, xlo = _split8(xs)
    xhi_r = xhi.reshape(NKD, P, T)
    xlo_r = xlo.reshape(NKD, P, T)
    xq = np.ascontiguousarray(
        np.concatenate([xlo_r, xhi_r], axis=0).reshape(2 * NKD * P, T))
    pi = np.arange(P)[:, None]
    kk = np.arange(P)[None, :]
    mask = np.where(kk <= pi, 0.0, -65504.0).astype(np.float16)
    mask = np.ascontiguousarray(mask)
    w_qkv = np.asarray(w_qkv, dtype=np.float32)
    b_qkv = np.asarray(b_qkv, dtype=np.float32)
    w_proj = np.asarray(w_proj, dtype=np.float32)
    b_proj = np.asarray(b_proj, dtype=np.float32)
    b2 = np.ascontiguousarray(
        (b_proj / NCORES).reshape(D // P, P).T).astype(np.float32)
    in_maps = []
    for c in range(NCORES):
        qcols = slice(c * DQC, (c + 1) * DQC)
        wsel = np.concatenate([w_qkv[:, qcols], w_qkv[:, D:]], axis=1) * SW
        whi, wlo = _split8(wsel)                  # [D, E1]
        # -> [eb, p, plane, kd, q]; plane0=hi
        w1 = np.stack([whi.reshape(NKD, P, NEB, P),
                       wlo.reshape(NKD, P, NEB, P)], axis=0)
        w1 = w1.transpose(3, 2, 0, 1, 4).reshape(NEB * P, 2 * NKD * P)
        b1 = SV * np.concatenate([b_qkv[qcols], b_qkv[D:]])
        wps = w_proj[c * DQC:(c + 1) * DQC, :] * SW
        w2hi, w2lo = _split8(wps)                 # [DQC, D]
        w2 = np.stack([w2hi.reshape(HC, P, D),
                       w2lo.reshape(HC, P, D)], axis=1).reshape(HC * 2 * P, D)
        in_maps.append({
            "xq": xq,
            "w1": np.ascontiguousarray(w1),
            "b1": np.ascontiguousarray(b1.reshape(NEB, P).T).astype(np.float32),
            "w2": np.ascontiguousarray(w2),
            "b2": b2,
            "mask": mask,
        })
    return in_maps


def kernel(hidden_states, w_qkv, b_qkv, w_proj, b_proj):
    nc = _get_program()
    in_maps = make_in_maps(hidden_states, w_qkv, b_qkv, w_proj, b_proj)
    res = run_bass_kernel_spmd(nc, in_maps, list(range(NCORES)))
    y = np.zeros((D, T), dtype=np.float32)
    for r in res.results:
        y += np.asarray(r["yt"]).astype(np.float32)
    return np.ascontiguousarray(y.T.reshape(B, S, D))


# revision 37
# speedup vs baseline: 1.0495x; 1.0017x over previous
"""GPTBigCode MQA causal attention block on 8 TRN2 NeuronCores — v4.

Tensor-parallel over heads (4 of 32 query heads per core, single KV head
replicated), row-parallel c_proj, bf16 partial outputs summed on host.

v4 = v3 (fp8 DoubleRow GEMMs + fp8 off-diag attention) plus:
- Scores in fp8 DoubleRow at 0.5x: K^T is plain fp8 (x16) duplicated
  across both stationary slots via a stride-0 AP; Q is split hi+lo fp8
  (x16) in the two moving slots, so one DoubleRow computes
  k8^T(q_hi+q_lo) — full Q precision, only K carries plain-fp8 error.
- Engine rebalance (GPSIMD cannot touch PSUM; DVE's 2x path needs all
  operands 2-byte): y evictions alternate DVE/ACT; SBUF-only fp8 ops
  (prob pair combine, q/v/at lo-splits) go to the idle GPSIMD; the
  softmax-denominator accumulation chain is fp16-only on DVE (2x mode)
  fed by per-pair GPSIMD combines that run in parallel.
"""

import numpy as np
from contextlib import ExitStack

import ml_dtypes
import concourse.bass as bass
import concourse.tile as tile
from concourse import bass_isa, mybir
from concourse.bass_utils import run_bass_kernel_spmd
from concourse.masks import make_identity

B, S, D = 2, 2048, 4096
H, DH = 32, 128
NCORES = 8
HC = H // NCORES          # 4 heads per core
DQC = HC * DH             # 512 q-dims per core
T = B * S                 # 4096 tokens
P = 128
NKD = D // P              # 32 contraction tiles in model dim
E1 = DQC + 2 * DH         # 768 per-core QKV output dims
NEB = E1 // P             # 6 e-blocks: 4 Q heads, K, V
QT = 512                  # tokens per (b,j) group
NJ = T // QT              # 8 groups
NJB = S // QT             # 4 groups per batch
SCALE = DH ** -0.5
NKC = NKD // 2            # kd tiles per xq chunk

SX = 32.0                 # x fp8 scale
SW = 2048.0               # weight fp8 scale
SV = 16.0                 # q/k/v scale (fp16 master + fp8)
DEQ = 1.0 / (SX * SW)     # 2^-16
EXPB = -2.0               # exp bias: p~ = e^(s*SCALE - 2)
NQKV = 48                 # DoubleRow instrs per QKV e-block

F32 = mybir.dt.float32
R32 = mybir.dt.float32r
BF16 = mybir.dt.bfloat16
F16 = mybir.dt.float16
FP8 = mybir.dt.float8e4
ACTF = mybir.ActivationFunctionType
DR = mybir.MatmulPerfMode.DoubleRow
ALU = mybir.AluOpType
NEG = -1.0e30
BF = ml_dtypes.bfloat16
E4 = ml_dtypes.float8_e4m3


def build_program():
    nc = bass.Bass()
    # plane-major fp8 x: row index = (plane*NKD + kd)*128 + p, plane0=lo/1=hi
    xq = nc.declare_dram_parameter("xq", [2 * NKD * P, T], FP8, isOutput=False)
    # w1: e-block-major, per e-block columns = (plane, kd, q), plane0=hi/1=lo
    w1 = nc.declare_dram_parameter("w1", [NEB * P, 2 * NKD * P], FP8,
                                   isOutput=False)
    b1 = nc.declare_dram_parameter("b1", [P, NEB], F32, isOutput=False)
    # w2: row index = (kh*2 + plane)*128 + p, plane0=hi/1=lo
    w2 = nc.declare_dram_parameter("w2", [HC * 2 * P, D], FP8, isOutput=False)
    b2 = nc.declare_dram_parameter("b2", [P, D // P], F32, isOutput=False)
    maskp = nc.declare_dram_parameter("mask", [P, P], F16, isOutput=False)
    yt = nc.declare_dram_parameter("yt", [D, T], BF16, isOutput=True)

    with tile.TileContext(nc) as tc:
        with ExitStack() as ctx:
            _body(ctx, tc, nc, xq, w1, b1, w2, b2, maskp, yt)
    _legalize_waits(nc)
    return nc


def _legalize_waits(nc, nop_cap=1):
    """walrus's per-instruction sync-wait budget is tiny for matmuls (LDW+MM
    lowering) and DMA pseudo-instructions. Drop redundant same-engine
    self-waits (engines execute in order), then spill excess waits onto
    same-engine NoOps inserted right before the instruction."""
    nocap = (mybir.InstNoOp,)
    f = nc.m.functions[0]
    for bb in f.blocks:
        insts = bb.instructions
        for i in insts:
            si = i.sync_info
            if si is None or not si.on_wait:
                continue
            ename = str(i.engine).split(".")[-1]
            if ename == "SP":
                ename = "Sync"
            kept = [w for w in si.on_wait
                    if w.sync_type != "semaphore"
                    or w.wait_reg is not None
                    or not w.ant_name.split("_")[0] == ename]
            if len(kept) != len(si.on_wait):
                si.on_wait = kept
        idx = 0
        while idx < len(insts):
            i = insts[idx]
            si = i.sync_info
            cap = None if isinstance(i, nocap) else 1
            if cap is not None and si is not None and len(si.on_wait) > cap:
                excess = list(si.on_wait[:-cap])
                si.on_wait = list(si.on_wait[-cap:])
                while excess:
                    chunk, excess = excess[:nop_cap], excess[nop_cap:]
                    nop = mybir.InstNoOp(
                        name=nc.get_next_instruction_name(), ins=[], outs=[])
                    nop.engine = i.engine
                    nop.sync_info = mybir.SyncInfo(on_wait=chunk, on_update=[])
                    nc.register_instruction(nop)
                    insts.insert(idx, nop)
                    idx += 1
            idx += 1


class _CProj:
    """Stepwise emitter for one q-block's c_proj, interleaved into the NEXT
    q-block's (exp-paced) attention. Per me-tile: 6 DoubleRow matmuls
    (2 hi@hi kh-pairs + 4 cross per-kh); eviction with fused 2^-16 dequant
    and bias alternates DVE / ACT to split the f32-psum read load."""

    LAG = 1

    def __init__(self, nc, tb, ati, w2_sb, b2_sb, yt3, ps_acc, y_pool,
                 final=False):
        self.nc = nc
        self.tb = tb
        self.final = final
        if final:
            self.LAG = 2
        self.ati = ati
        self.w2_sb = w2_sb
        self.b2_sb = b2_sb
        self.yt3 = yt3
        self.ps_acc = ps_acc
        self.y_pool = y_pool
        self.ps_ys = {}
        self.y_t = None
        self.done = 0
        self.hdone = 0
        self.total = D // P + self.LAG

    def step(self):
        return self.half_step() and (self.half_step() or True)

    def half_step(self):
        if self.hdone >= 2 * self.total:
            return False
        me, phase = self.hdone // 2, self.hdone % 2
        self.hdone += 1
        self.done = self.hdone // 2
        nc = self.nc
        NME = D // P
        MG = NME // 8
        if phase == 0 and me >= self.LAG:
            md = me - self.LAG
            ps_y = self.ps_ys.pop(md)
            nc.tensor.matmul(ps_y[:],
                             self.w2_sb[:, HC - 1, :, md * P:(md + 1) * P],
                             self.ati[:, HC - 1, :, :],
                             start=False, stop=True, perf_mode=DR)
            mg, mi = md // MG, md % MG
            if mi == 0:
                y_t = self.y_pool.tile([P, MG, QT], BF16, tag="y")
                self.y_t = y_t
            if True:
                nc.vector.tensor_scalar(self.y_t[:, mi, :], ps_y[:],
                                        DEQ, self.b2_sb[:, md:md + 1],
                                        ALU.mult, ALU.add)
            else:
                nc.scalar.activation(self.y_t[:, mi, :], ps_y[:],
                                     ACTF.Identity, scale=DEQ,
                                     bias=self.b2_sb[:, md:md + 1])
            if self.final and mg == NME // MG - 1:
                nc.sync.dma_start(
                    out=self.yt3[:, mg * MG + mi:mg * MG + mi + 1,
                                 self.tb:self.tb + QT],
                    in_=self.y_t[:, mi:mi + 1, :])
            elif mi == MG - 1:
                nc.sync.dma_start(
                    out=self.yt3[:, mg * MG:(mg + 1) * MG,
                                 self.tb:self.tb + QT],
                    in_=self.y_t[:])
        if phase == 1 and me < NME:
            ps_y = self.ps_acc.tile([P, QT], F32, tag="acc")
            self.ps_ys[me] = ps_y
            cols = slice(me * P, (me + 1) * P)
            nc.tensor.matmul(ps_y[:], self.w2_sb[:, 0:2, 0, cols],
                             self.ati[:, 0:2, 1, :],
                             start=True, stop=False, perf_mode=DR)
            nc.tensor.matmul(ps_y[:], self.w2_sb[:, 2:4, 0, cols],
                             self.ati[:, 2:4, 1, :],
                             start=False, stop=False, perf_mode=DR)
            for kh in range(HC - 1):
                nc.tensor.matmul(ps_y[:], self.w2_sb[:, kh, :, cols],
                                 self.ati[:, kh, :, :],
                                 start=False, stop=False, perf_mode=DR)
        return True


def _body(ctx, tc, nc, xq, w1, b1, w2, b2, maskp, yt):
    xq4 = xq.rearrange("(two kd p) t -> p two kd t", p=P, two=2)
    w13 = w1.rearrange("(eb p) x -> p eb x", p=P)
    w24 = w2.rearrange("(kh two p) d -> p kh two d", p=P, two=2)
    yt3 = yt.rearrange("(me p) t -> p me t", p=P)

    persist = ctx.enter_context(tc.tile_pool(name="persist", bufs=1))
    w1_sb = persist.tile([P, NEB, 2, NKD, P], FP8)   # [d_in, eb, hi/lo, kd, q]
    w2_sb = persist.tile([P, HC, 2, D], FP8)         # [dqc, kh, hi/lo, d_out]
    kt_sb = persist.tile([P, T], FP8)                # 16*K^T [dh, t] plain fp8
    v16_sb = persist.tile([P, T // P, DH], F16)      # 16*V [t_part, mt, dh]
    vio_sb = persist.tile([P, T // P, 2, DH], FP8)   # V fp8 (hi, lo)
    b1_sb = persist.tile([P, NEB], F32)
    b2_sb = persist.tile([P, D // P], F32)
    mask16 = persist.tile([P, P], F16)   # causal mask^T (0/-65504), PE-applied
    nbias = persist.tile([P, 1], F32)                # exp bias -2
    ones_mat = persist.tile([P, P], F16)             # 0.5: folds scales
    ones8 = persist.tile([P, P], FP8)                # 0.5 for fp8 den DR
    ones8q = persist.tile([P, P], FP8)               # 0.25: dup-slot den DR
    ident = persist.tile([P, P], F16)
    nc.vector.memset(ones_mat[:], 0.5)
    nc.vector.memset(ones8[:], 0.5)
    nc.vector.memset(ones8q[:], 0.25)
    nc.vector.memset(nbias[:], EXPB)

    # w1 (per e-block) and the first q-block's xq are queued in the order
    # the first QKV e-block consumes them.
    xt_pool = ctx.enter_context(tc.tile_pool(name="xt", bufs=3))
    xt_first = []
    for _half in range(2):
        xt_c = xt_pool.tile([P, 2, NKC, QT], FP8, tag="xt")
        xt_first.append(xt_c)
    W1C = 2 * NKD * P
    # (kind, eb/chunk, plane): eb0's hi weights + both chunks' hi planes
    # first, so the 16 hi@hi matmuls of eb0 start after ~1.5MB of DMA.
    for kind, a, pl in [
            ('w1', 0, 0), ('xt', 0, 1), ('xt', 1, 1), ('w1', 0, 1),
            ('xt', 0, 0), ('xt', 1, 0), ('w1', 1, 0), ('w1', 1, 1),
            ('w1', 2, None), ('w1', 3, None), ('w1', 4, None),
            ('w1', 5, None)]:
        if kind == 'w1':
            if pl is None:
                nc.sync.dma_start(
                    out=w1_sb[:, a].rearrange("p two kd q -> p (two kd q)"),
                    in_=w13[:, a, :])
            else:
                nc.sync.dma_start(
                    out=w1_sb[:, a, pl], in_=w13[:, a, pl * (W1C // 2):
                                                 (pl + 1) * (W1C // 2)]
                    .rearrange("p (kd q) -> p kd q", q=P))
        else:
            nc.scalar.dma_start(
                out=xt_first[a][:, pl, :, :],
                in_=xq4[:, pl, a * NKC:(a + 1) * NKC, 0:QT])
    nc.scalar.dma_start(out=b1_sb[:], in_=b1[:])
    nc.scalar.dma_start(out=mask16[:], in_=maskp[:])
    make_identity(nc, ident[:])

    # PSUM: 3 (acc) + 2*2 (score pairs) + 1 (misc) = 8 banks
    ps_acc = ctx.enter_context(tc.tile_pool(name="ps_acc", bufs=3, space="PSUM"))
    ps_pair = ctx.enter_context(tc.tile_pool(name="ps_pair", bufs=2, space="PSUM"))
    ps_misc = ctx.enter_context(tc.tile_pool(name="ps_misc", bufs=1, space="PSUM"))

    qt_pool = ctx.enter_context(tc.tile_pool(name="qt", bufs=2))
    q16_pool = ctx.enter_context(tc.tile_pool(name="q16", bufs=2))
    vs_pool = ctx.enter_context(tc.tile_pool(name="vs", bufs=2))
    p_pool = ctx.enter_context(tc.tile_pool(name="pp", bufs=4))
    p16_pool = ctx.enter_context(tc.tile_pool(name="p16", bufs=3))
    psum_pool = ctx.enter_context(tc.tile_pool(name="psm", bufs=2))
    ibc_pool = ctx.enter_context(tc.tile_pool(name="ibc", bufs=2))
    at16_pool = ctx.enter_context(tc.tile_pool(name="a16", bufs=2))
    ati_pool = ctx.enter_context(tc.tile_pool(name="ati", bufs=3))
    y_pool = ctx.enter_context(tc.tile_pool(name="yp", bufs=2))

    class _QKV:
        """Stepwise emitter for one q-block's QKV: per e-block, 16 hi@hi
        DoubleRow (adjacent kd pairs, hi planes) + 32 cross DoubleRow
        (per-kd (w_hi,x_lo)+(w_lo,x_hi)) into one PSUM group."""

        def __init__(self, j, xt_cs):
            self.tb = j * QT
            self.xt_cs = xt_cs
            self.qi = qt_pool.tile([P, HC, 2, QT], FP8, tag="qt")
            self.v_st = None
            self.eb = 0
            self.mi = 0
            self.ps = None
            self.total_mm = NEB * NQKV
            self.done_mm = 0

        def step(self, n_mm=8):
            if self.eb >= NEB:
                return False
            for _ in range(n_mm):
                if self.ps is None:
                    self.ps = ps_acc.tile([P, QT], F32, tag="acc")
                eb, mi = self.eb, self.mi
                if mi < NKD // 2:          # hi@hi: kd pair (2mi, 2mi+1)
                    kd0 = 2 * mi
                    c, r = kd0 // NKC, kd0 % NKC
                    nc.tensor.matmul(
                        self.ps[:], w1_sb[:, eb, 0, kd0:kd0 + 2, :],
                        self.xt_cs[c][:, 1, r:r + 2, :],
                        start=(mi == 0), stop=False, perf_mode=DR)
                else:                      # cross: kd = mi - 16
                    kd = mi - NKD // 2
                    c, r = kd // NKC, kd % NKC
                    nc.tensor.matmul(
                        self.ps[:], w1_sb[:, eb, :, kd, :],
                        self.xt_cs[c][:, :, r, :],
                        start=False, stop=(mi == NQKV - 1), perf_mode=DR)
                self.done_mm += 1
                self.mi += 1
                if self.mi == NQKV:
                    self._evict()
                    self.mi = 0
                    self.eb += 1
                    self.ps = None
                    if self.eb >= NEB:
                        return False
            return True

        def _evict(self):
            eb, ps = self.eb, self.ps
            # b1 is pre-scaled x16 on host for all columns
            if eb < HC:      # Q head: 16*q -> f16 master, then fp8 hi+lo
                q16 = q16_pool.tile([P, QT], F16, tag="q16")
                nc.scalar.activation(q16[:], ps[:],
                                     ACTF.Identity, scale=DEQ * SV,
                                     bias=b1_sb[:, eb:eb + 1])
                nc.scalar.copy(self.qi[:, eb, 1, :], q16[:])
                nc.gpsimd.tensor_sub(self.qi[:, eb, 0, :], q16[:],
                                     self.qi[:, eb, 1, :])
            elif eb == HC:   # K^T: plain fp8 x16
                nc.scalar.activation(kt_sb[:, self.tb:self.tb + QT], ps[:],
                                     ACTF.Identity, scale=DEQ * SV,
                                     bias=b1_sb[:, eb:eb + 1])
            else:            # V: 16*(v+b) -> f16 on DVE
                v_s = vs_pool.tile([P, QT], F16, tag="vs")
                nc.vector.tensor_scalar(v_s[:], ps[:], DEQ * SV,
                                        b1_sb[:, eb:eb + 1],
                                        ALU.mult, ALU.add)
                self.v_st = v_s

    def prefetch_xq(jp):
        cs = []
        for half in range(2):
            xt_c = xt_pool.tile([P, 2, NKC, QT], FP8, tag="xt")
            for pl in (1, 0):
                nc.sync.dma_start(
                    out=xt_c[:, pl, :, :],
                    in_=xq4[:, pl, half * NKC:(half + 1) * NKC,
                             jp * QT:jp * QT + QT])
            cs.append(xt_c)
        return cs

    SPILL = 16     # half-steps of c_proj spilled into the next (jj=3) window
    cproj_q = []
    qkv_cur = None
    qkv_next = None
    xt_next = None
    for j in range(NJ):
        b, jj = j // NJB, j % NJB
        tb = j * QT

        # ---- QKV for tokens [tb, tb+QT) -----------------------------------
        qkv_cur = qkv_next if qkv_next is not None else _QKV(j, xt_first)
        qkv_next = None
        if j + 1 < NJ and j > 0:
            xt_next = prefetch_xq(j + 1)
        while qkv_cur.step():
            pass
        qi = qkv_cur.qi
        v_st = qkv_cur.v_st

        # ---- attention for this q-block (4 heads) -------------------------
        # Scores: one DoubleRow per k-tile (K^T stride-0-duplicated in the
        # stationary slots, q hi+lo in the moving slots). Off-diag pairs:
        # one fp8 exp covers both k-tiles, then 2 DoubleRow PV (vhi, vlo
        # slot-paired across the pair). Diagonal: fp16 probs and fp16 V.
        # Denominator: per-pair GPSIMD combine (fp8+fp8->f16) + fp16-only
        # DVE accumulate chain (2x mode); one 0.5-matmul per head reduces
        # and broadcasts it with the x16 V / x32 at scales folded in.
        ati = ati_pool.tile([P, HC, 2, QT], FP8, tag="ati")  # plane0=lo/1=hi
        nk = 4 * jj + 4
        units = [(kk, kk + 1) for kk in range(0, 4 * jj, 2)] \
            + [(kk,) for kk in range(4 * jj, nk)]

        def emit_unit(h, u):
            kks = units[u]
            psp = ps_pair.tile([P, 2, QT], F32, tag="pair")
            if len(kks) == 2:
                p8 = p_pool.tile([P, 2, QT], FP8, tag="p")
                for i, kk in enumerate(kks):
                    c0 = b * S + kk * P
                    k_dup = (kt_sb[:, c0:c0 + P]
                             .rearrange("p (one q) -> p one q", one=1)
                             .broadcast_to([P, 2, P]))
                    nc.tensor.matmul(psp[:, i, :], k_dup, qi[:, h, :, :],
                                     start=True, stop=True, perf_mode=DR)
                nc.scalar.activation(p8[:, :, :], psp[:, :, :],
                                     ACTF.Exp, scale=SCALE / (SV * SV),
                                     bias=nbias[:])
                return ('off', p8, kks[0], None)
            kk = kks[0]
            qoff = P * (kk - 4 * jj)
            c0 = b * S + kk * P
            k_dup = (kt_sb[:, c0:c0 + P]
                     .rearrange("p (one q) -> p one q", one=1)
                     .broadcast_to([P, 2, P]))
            nc.tensor.matmul(psp[:, 0, qoff:], k_dup, qi[:, h, :, qoff:],
                             start=True, stop=False, perf_mode=DR)
            # causal mask for the diagonal 128x128: one f16 matmul
            # (mask^T stationary x identity) accumulates 0/-65504 into the
            # scores -- keeps the mask off the DVE and out of its queue
            nc.tensor.matmul(psp[:, 0, qoff:qoff + P], mask16[:], ident[:],
                             start=False, stop=True)
            if jj > 0 or kk > 0:
                # fp8-fragile rows (few prior keys) are only touched by the
                # kk==0 diagonal tile; every other tile's rows have >=128
                # prior keys, so fp8 probs are safe and PV/den can use
                # DoubleRow like the off-diagonal tiles
                p8d = p_pool.tile([P, QT], FP8, tag="p8d")
                nc.scalar.activation(p8d[:, qoff:], psp[:, 0, qoff:],
                                     ACTF.Exp, scale=SCALE / (SV * SV),
                                     bias=nbias[:])
                return ('diag8', p8d, kk, qoff)
            p16 = p16_pool.tile([P, QT], F16, tag="p16")
            nc.scalar.activation(p16[:, qoff:], psp[:, 0, qoff:],
                                 ACTF.Exp, scale=SCALE / (SV * SV),
                                 bias=nbias[:])
            return ('diag', p16, kk, qoff)

        def finalize_head(h, ps_out, p_sum, ps_db, had_pairs):
            # jj==0: final 0.5-matmul folds the fp16 diagonal prob sums into
            # the head's f32 den accumulator; jj>=1 accumulated everything
            # on the PE already (pair ones8-DR + diag dup-slot ones8q-DR).
            if p_sum is not None:
                nc.tensor.matmul(ps_db[:], ones_mat[:], p_sum[:],
                                 start=not had_pairs, stop=True)
            inv_bc = ibc_pool.tile([P, QT], F16, tag="ibc")
            with nc.allow_low_precision(reason="f16 inv: den spans 2e-3..500, "
                                        "1e-3 rel err ≪ fp8 prob noise"):
                nc.vector.reciprocal(inv_bc[:], ps_db[:])
            at16 = at16_pool.tile([P, QT], F16, tag="a16")
            nc.vector.tensor_mul(at16[:], ps_out[:], inv_bc[:])
            nc.gpsimd.tensor_copy(ati[:, h, 1, :], at16[:])
            nc.gpsimd.tensor_sub(ati[:, h, 0, :], at16[:], ati[:, h, 1, :])

        NU = len(units)
        stream = [(h, u) for h in range(HC) for u in range(NU)]
        total_units = len(stream)
        units_done = 0
        cq_done = sum(cp.hdone for cp in cproj_q)
        cq_start = cq_done
        cq_budget = sum(2 * cp.total for cp in cproj_q) - cq_start
        pending = None
        ps_out = None
        p_sum = None
        u_next = emit_unit(*stream[0])
        # V transposes (fp16) for this q-block, then fp8 hi/lo planes
        for i in range(QT // P):
            tp = ps_acc.tile([P, P], F16, tag="acc")
            nc.tensor.transpose(tp[:], v_st[:, i * P:(i + 1) * P],
                                ident[:])
            mt = j * (QT // P) + i
            nc.vector.tensor_copy(v16_sb[:, mt, :], tp[:])
            nc.scalar.copy(vio_sb[:, mt, 0, :], v16_sb[:, mt, :])
            nc.gpsimd.tensor_sub(vio_sb[:, mt, 1, :], v16_sb[:, mt, :],
                                 vio_sb[:, mt, 0, :])
        if j + 1 < NJ:
            if j == 0:
                xt_next = prefetch_xq(1)
                # w2 is first needed by cproj0 (next block's attention);
                # issuing it after block 1's xq keeps the serial DMA stream
                # feeding the QKV filler first
                nc.scalar.dma_start(out=w2_sb[:], in_=w24[:])
                nc.scalar.dma_start(out=b2_sb[:], in_=b2[:])
            qkv_next = _QKV(j + 1, xt_next)
        ps_db = None
        had_pairs = False
        for idx, (h, u) in enumerate(stream):
            kind, pt, kk0, extra = u_next
            if u == 0 and pending is not None:
                finalize_head(*pending)
                pending = None
            if idx + 1 < total_units:
                u_next = emit_unit(*stream[idx + 1])
            if u == 0:
                ps_out = ps_acc.tile([P, QT], F32, tag="acc")
                if jj == 0:
                    p_sum = psum_pool.tile([P, QT], F16, tag="psum")
                else:
                    p_sum = None
                ps_db = ps_misc.tile([P, QT], F32, tag="misc")
                had_pairs = False
            # filler BEFORE this unit's PV matmuls (cover the exp latency
            # the PV waits on): previous block's c_proj, then the next
            # block's QKV
            if cproj_q:
                target = cq_start + cq_budget * (units_done + 3) // total_units
                while cq_done < target and cproj_q:
                    if cproj_q[0].half_step():
                        cq_done += 1
                    else:
                        cproj_q.pop(0)
            if qkv_next is not None and units_done > 0:
                target = qkv_next.total_mm * (units_done + 2) // total_units
                while qkv_next.done_mm < target and qkv_next.step(8):
                    pass
            if kind == 'off':
                mt0 = b * (S // P) + kk0
                nc.tensor.matmul(ps_out[:], vio_sb[:, mt0:mt0 + 2, 0, :],
                                 pt[:, :, :], start=(kk0 == 0), stop=False,
                                 perf_mode=DR)
                nc.tensor.matmul(ps_out[:], vio_sb[:, mt0:mt0 + 2, 1, :],
                                 pt[:, :, :], start=False, stop=False,
                                 perf_mode=DR)
                o_dup = (ones8[:]
                         .rearrange("p (one q) -> p one q", one=1)
                         .broadcast_to([P, 2, P]))
                nc.tensor.matmul(ps_db[:], o_dup, pt[:, :, :],
                                 start=not had_pairs, stop=False,
                                 perf_mode=DR)
                had_pairs = True
            elif kind == 'diag8':
                kk, qoff = kk0, extra
                mt = b * (S // P) + kk
                nw = QT - qoff
                p_dup = (pt[:, qoff:]
                         .rearrange("p (one q) -> p one q", one=1)
                         .broadcast_to([P, 2, nw]))
                nc.tensor.matmul(ps_out[:, qoff:], vio_sb[:, mt, :, :],
                                 p_dup, start=False, stop=(kk == nk - 1),
                                 perf_mode=DR)
                oq_dup = (ones8q[:]
                          .rearrange("p (one q) -> p one q", one=1)
                          .broadcast_to([P, 2, P]))
                if not had_pairs:
                    # jj==0 head: this den-DR opens the ps_db group; zero
                    # the unwritten prefix and cover the full width
                    nc.vector.memset(pt[:, 0:qoff], 0.0)
                    pd_full = (pt[:]
                               .rearrange("p (one q) -> p one q", one=1)
                               .broadcast_to([P, 2, QT]))
                    nc.tensor.matmul(ps_db[:], oq_dup, pd_full,
                                     start=True, stop=(kk == nk - 1),
                                     perf_mode=DR)
                else:
                    nc.tensor.matmul(ps_db[:, qoff:], oq_dup, p_dup,
                                     start=False, stop=(kk == nk - 1),
                                     perf_mode=DR)
                had_pairs = True
            else:
                kk, qoff = kk0, extra
                nc.tensor.matmul(ps_out[:, qoff:],
                                 v16_sb[:, b * (S // P) + kk, :],
                                 pt[:, qoff:], start=(kk == 0),
                                 stop=(kk == nk - 1))
                if kk == 4 * jj:
                    nc.vector.tensor_copy(p_sum[:], pt[:])
                else:
                    nc.vector.tensor_add(p_sum[:, qoff:], p_sum[:, qoff:],
                                         pt[:, qoff:])
            units_done += 1
            if u == NU - 1:
                pending = (h, ps_out, p_sum, ps_db, had_pairs)
        finalize_head(*pending)
        keep = SPILL if (j + 1 < NJ and (j + 1) % NJB == NJB - 1) else 0
        for ci, cp in enumerate(cproj_q):
            limit = 2 * cp.total - (keep if ci == len(cproj_q) - 1 else 0)
            while cp.hdone < limit and cp.half_step():
                pass
        cproj_q = [cp for cp in cproj_q if cp.hdone < 2 * cp.total]
        cproj_q.append(_CProj(nc, tb, ati, w2_sb, b2_sb, yt3,
                              ps_acc, y_pool, final=(j == NJ - 1)))
    for cp in cproj_q:
        while cp.step():
            pass


_PROGRAM = None


def _get_program():
    global _PROGRAM
    if _PROGRAM is None:
        _PROGRAM = build_program()
    return _PROGRAM


def _split8(a):
    hi = a.astype(E4)
    lo = (a - hi.astype(np.float32)).astype(E4)
    return hi, lo


def make_in_maps(hidden_states, w_qkv, b_qkv, w_proj, b_proj):
    x = np.asarray(hidden_states, dtype=np.float32).reshape(T, D)
    xs = np.ascontiguousarray(x.T) * SX          # [D, T]
    xhi, xlo = _split8(xs)
    xhi_r = xhi.reshape(NKD, P, T)
    xlo_r = xlo.reshape(NKD, P, T)
    xq = np.ascontiguousarray(
        np.concatenate([xlo_r, xhi_r], axis=0).reshape(2 * NKD * P, T))
    pi = np.arange(P)[:, None]
    kk = np.arange(P)[None, :]
    mask = np.where(kk <= pi, 0.0, -65504.0).astype(np.float16)
    mask = np.ascontiguousarray(mask)
    w_qkv = np.asarray(w_qkv, dtype=np.float32)
    b_qkv = np.asarray(b_qkv, dtype=np.float32)
    w_proj = np.asarray(w_proj, dtype=np.float32)
    b_proj = np.asarray(b_proj, dtype=np.float32)
    b2 = np.ascontiguousarray(
        (b_proj / NCORES).reshape(D // P, P).T).astype(np.float32)
    in_maps = []
    for c in range(NCORES):
        qcols = slice(c * DQC, (c + 1) * DQC)
        wsel = np.concatenate([w_qkv[:, qcols], w_qkv[:, D:]], axis=1) * SW
        whi, wlo = _split8(wsel)                  # [D, E1]
        # -> [eb, p, plane, kd, q]; plane0=hi
        w1 = np.stack([whi.reshape(NKD, P, NEB, P),
                       wlo.reshape(NKD, P, NEB, P)], axis=0)
        w1 = w1.transpose(3, 2, 0, 1, 4).reshape(NEB * P, 2 * NKD * P)
        b1 = SV * np.concatenate([b_qkv[qcols], b_qkv[D:]])
        wps = w_proj[c * DQC:(c + 1) * DQC, :] * SW
        w2hi, w2lo = _split8(wps)                 # [DQC, D]
        w2 = np.stack([w2hi.reshape(HC, P, D),
                       w2lo.reshape(HC, P, D)], axis=1).reshape(HC * 2 * P, D)
        in_maps.append({
            "xq": xq,
            "w1": np.ascontiguousarray(w1),
            "b1": np.ascontiguousarray(b1.reshape(NEB, P).T).astype(np.float32),
            "w2": np.ascontiguousarray(w2),
            "b2": b2,
            "mask": mask,
        })
    return in_maps


def kernel(hidden_states, w_qkv, b_qkv, w_proj, b_proj):
    nc = _get_program()
    in_maps = make_in_maps(hidden_states, w_qkv, b_qkv, w_proj, b_proj)
    res = run_bass_kernel_spmd(nc, in_maps, list(range(NCORES)))
    y = np.zeros((D, T), dtype=np.float32)
    for r in res.results:
        y += np.asarray(r["yt"]).astype(np.float32)
    return np.ascontiguousarray(y.T.reshape(B, S, D))
